# revision 1
# baseline (speedup 1.0000x reference)
"""Trainium2 Bass kernel for nn_Encoder_6 (conv+GN+InterpLnr x3 -> biLSTM).

Self-contained: host-side prep (sharding, interp gather tables, weight
repacking) + Bass/Tile device kernel + output gather.

Data-parallel over 8 NeuronCores: 64 samples per core.

Device dataflow per core (all samples resident on-chip after one load):
  - activations live in [channel(partition), sample, time] layout
  - conv1d = 10-11 accumulating matmuls per sample-pair (taps x cin-chunks),
    PSUM [128, 2x192]
  - GroupNorm stats fused into PSUM evacuation (ACT copy+accum -> sums,
    DVE square+accum -> sumsq), group reduce + expand via tiny matmuls
  - normalize+ReLU = single ACT op (per-partition scale/bias)
  - InterpLnr = banded-matrix matmul per sample (layer 2 also emits a
    time-reversed copy for the backward LSTM direction)
  - biLSTM in gate-major layout: state tiles are [64=(dir,unit), sample];
    per step: 4 tiny recurrence matmuls into the PSUM gate block, one
    sigmoid ACT over (i,f,2g) [tanh(g)=2*sig(2g)-1, the 2x folded into
    host weights], one sigmoid ACT for o, 3 DVE ops for the cell update,
    tanh ACT, 1 DVE for h.  No per-step transposes.
"""
import sys
from contextlib import ExitStack

sys.path.insert(0, "/opt/trn_rl_repo")

import numpy as np
import ml_dtypes

B = 512
N_CORES = 8
S = B // N_CORES          # samples per core
DIM_PIT = 257
C = 256                   # conv channels
T = 192                   # padded time
TH = 196                  # time with halo (2 each side)
GRP = 16                  # channels per group
DIM_NECK = 32
FREQ = 8
NT_OUT = 24               # output timesteps per direction
MIN_LEN_SEG = 19
MAX_NUM_SEG = 7
W64 = 64                  # 2*MAX_LEN_SEG
EPS = 1e-5
SG = 32                   # samples per stats group (2 groups per core)
NPAIR = 16                # sample pairs per stats group
LBLK = 2                  # LSTM timesteps per PSUM block (block = 1 PSUM bank)
NBLK = T // LBLK

_cache = {}


# ---------------------------------------------------------------- host prep

def _interp_tables(scales_u, len_seg_raw, n):
    """Gather idx/w1/w2 per sample for one interp layer (numpy, exact)."""
    scales = scales_u.astype(np.float32) + np.float32(0.5)
    j = np.arange(W64, dtype=np.float32)
    idx_scaled = j[None, :] / scales[:, None]
    idx_fl = np.floor(idx_scaled)
    lam = idx_scaled - idx_fl
    len_seg = (len_seg_raw + MIN_LEN_SEG).astype(np.float32)[:, None]
    idx_mask = idx_fl < (len_seg - 1.0)
    ls = (len_seg_raw + MIN_LEN_SEG).reshape(n, MAX_NUM_SEG)
    offset = np.cumsum(ls, axis=-1)
    offset = np.pad(offset[:, :-1], ((0, 0), (1, 0))).reshape(-1, 1)
    idx_org = idx_fl + offset.astype(np.float32)
    mask = (idx_mask & (idx_org < (T - 1))).reshape(n, MAX_NUM_SEG * W64)
    idx_b = np.clip(idx_org.reshape(n, -1).astype(np.int32), 0, T - 2)
    lam_b = lam.reshape(n, -1)
    idx = np.zeros((n, T), np.int32)
    w1 = np.zeros((n, T), np.float32)
    w2 = np.zeros((n, T), np.float32)
    for b in range(n):
        js = np.nonzero(mask[b])[0][:T]
        k = len(js)
        idx[b, :k] = idx_b[b, js]
        w1[b, :k] = 1.0 - lam_b[b, js]
        w2[b, :k] = lam_b[b, js]
    return idx, w1, w2


def _prep_host(inputs):
    """Build per-core input dicts. Returns list of 8 dicts."""
    x = np.asarray(inputs["x"], np.float32)
    scales = np.asarray(inputs["scales"], np.float32)
    lsr = np.asarray(inputs["len_seg_raw"], np.int32)

    # conv weights as lhsT tiles [l, chunk, tap, half, cin128, cout128]
    wconv = np.zeros((3, 2, 5, 2, 128, 128), np.float32)
    for l in range(3):
        w = np.asarray(inputs[f"conv{l}_w"], np.float32)  # [256, cin, 5]
        for cc in range(2):
            for k in range(5):
                for h in range(2):
                    wconv[l, cc, k, h] = w[h * 128:(h + 1) * 128,
                                           cc * 128:(cc + 1) * 128, k].T
    wconv = np.ascontiguousarray(wconv.astype(np.float16))
    # conv0 channel 256 as [5, 256] lhsT (k=tap)
    w0 = np.asarray(inputs["conv0_w"], np.float32)
    wc0e = np.ascontiguousarray(w0[:, 256, :].T.astype(np.float16))  # [5, 256]

    conv_bias = [np.asarray(inputs[f"conv{l}_b"], np.float32) for l in range(3)]
    assert all(np.abs(b).max() == 0.0 for b in conv_bias), \
        "nonzero conv bias not implemented in device kernel"

    gamma_t = np.stack([np.asarray(inputs[f"gn{l}_g"], np.float32).reshape(2, 128)
                        for l in range(3)])          # [3, 2, 128]
    beta_t = np.stack([np.asarray(inputs[f"gn{l}_b"], np.float32).reshape(2, 128)
                       for l in range(3)])
    gamma_t = np.ascontiguousarray(gamma_t.transpose(2, 0, 1).reshape(128, 6))
    beta_t = np.ascontiguousarray(beta_t.transpose(2, 0, 1).reshape(128, 6))

    gind = np.zeros((128, 8), np.float32)
    for c in range(128):
        gind[c, c // 16] = 1.0
    gexp = np.ascontiguousarray(gind.T)               # [8, 128]

    # interp tables, all samples
    idx_all, w1_all, w2_all = [], [], []
    for l in range(3):
        idx, w1, w2 = _interp_tables(scales[l], lsr[l], B)
        idx_all.append(idx)
        w1_all.append(w1)
        w2_all.append(w2)

    # LSTM weights, gate-major layout. gate order i,f,g,o; g-gate scaled
    # by 2 (tanh(g) = 2*sigmoid(2g) - 1 on device).
    #  wihG [128 cin, 4 gate, 2 cc, 64 (d,u)]  lhsT of xw matmuls
    #  whhG [64 (d,u'), 4 gate, 64 (d,u)]      lhsT of recurrence matmuls
    #  biasG [4 gate, 64 (d,u)]                lhsT of rank-1 bias matmuls
    H = DIM_NECK
    wihG = np.zeros((128, 4, 2, 64), np.float32)
    whhG = np.zeros((65, 4, 64), np.float32)   # row 64 = bias (ones in rhs)
    for d, nm in enumerate(["f", "b"]):
        wi = np.asarray(inputs[f"w_ih_{nm}"], np.float32)   # [128, 256]
        wh = np.asarray(inputs[f"w_hh_{nm}"], np.float32)   # [128, 32]
        bb = (np.asarray(inputs[f"b_ih_{nm}"], np.float32)
              + np.asarray(inputs[f"b_hh_{nm}"], np.float32))
        for g in range(4):
            sc = 2.0 if g == 2 else 1.0
            for cc in range(2):
                wihG[:, g, cc, d * H:(d + 1) * H] = \
                    sc * wi[g * H:(g + 1) * H, cc * 128:(cc + 1) * 128].T
            whhG[d * H:(d + 1) * H, g, d * H:(d + 1) * H] = \
                sc * wh[g * H:(g + 1) * H, :].T
            whhG[64, g, d * H:(d + 1) * H] = sc * bb[g * H:(g + 1) * H]
    wihG = np.ascontiguousarray(wihG.astype(np.float16))
    whhG = np.ascontiguousarray(whhG.astype(np.float16))

    in_maps = []
    for core in range(N_CORES):
        s0 = core * S
        xs = x[s0:s0 + S]                              # [S, 257, 192]
        xt = xs.transpose(1, 0, 2)                     # [257, S, 192]
        xab = np.zeros((128, 2, S, TH), np.float32)
        xab[:, 0, :, 2:194] = xt[:128]
        xab[:, 1, :, 2:194] = xt[128:256]
        xc = np.zeros((5, S, T), np.float32)
        x256 = xt[256]                                 # [S, 192]
        for k in range(5):
            sh = k - 2
            lo, hi = max(0, -sh), min(T, T - sh)
            xc[k, :, lo:hi] = x256[:, lo + sh:hi + sh]

        # banded interp matrices S[t_in, t_out] per (layer, sample), fp16
        wS = np.zeros((3, S, T, T), np.float16)
        bi = np.arange(S)[:, None]
        pj = np.arange(T)[None, :]
        for l in range(3):
            idx = idx_all[l][s0:s0 + S]
            Sm = np.zeros((S, T, T), np.float32)
            Sm[bi, idx, pj] = w1_all[l][s0:s0 + S]
            Sm[bi, idx + 1, pj] += w2_all[l][s0:s0 + S]
            wS[l] = Sm.astype(np.float16)

        in_maps.append({
            "xab": np.ascontiguousarray(xab.astype(np.float16)),
            "xc": np.ascontiguousarray(xc.astype(np.float16)),
            "wconv": wconv,
            "wc0e": wc0e,
            "gamma_t": gamma_t,
            "beta_t": beta_t,
            "gind": gind,
            "gexp": gexp,
            "wS": np.ascontiguousarray(wS),
            "id128": np.eye(128, dtype=np.float16),
            "wihG": wihG,
            "whhG": whhG,
        })
    return in_maps


# ------------------------------------------------------------- device build

def _build(probe_layer=-1):
    """Build the Bacc module. probe_layer >= 0 adds a probe output of XBUF
    after that layer's interp (for debugging)."""
    import concourse.bass as bass
    import concourse.tile as tile
    from concourse import bacc, mybir
    from concourse.masks import make_identity

    f32 = mybir.dt.float32
    f32r = mybir.dt.float32r
    bf16 = mybir.dt.bfloat16
    fp16 = mybir.dt.float16
    AF = mybir.ActivationFunctionType
    OP = mybir.AluOpType

    nc = bacc.Bacc("TRN2", target_bir_lowering=False, debug=False,
                   enable_asserts=False, num_devices=N_CORES)

    # DRAM tensors
    d_xab = nc.dram_tensor("xab", [128, 2, S, TH], fp16, kind="ExternalInput")
    d_xc = nc.dram_tensor("xc", [5, S, T], fp16, kind="ExternalInput")
    d_wconv = nc.dram_tensor("wconv", [3, 2, 5, 2, 128, 128], fp16,
                             kind="ExternalInput")
    d_wc0e = nc.dram_tensor("wc0e", [5, 256], fp16, kind="ExternalInput")
    d_gamma = nc.dram_tensor("gamma_t", [128, 6], f32, kind="ExternalInput")
    d_beta = nc.dram_tensor("beta_t", [128, 6], f32, kind="ExternalInput")
    d_gind = nc.dram_tensor("gind", [128, 8], f32, kind="ExternalInput")
    d_gexp = nc.dram_tensor("gexp", [8, 128], f32, kind="ExternalInput")
    d_wS = nc.dram_tensor("wS", [3, S, T, T], fp16, kind="ExternalInput")
    d_id128 = nc.dram_tensor("id128", [128, 128], fp16, kind="ExternalInput")
    d_wihG = nc.dram_tensor("wihG", [128, 4, 2, 64], fp16,
                            kind="ExternalInput")
    d_whhG = nc.dram_tensor("whhG", [65, 4, 64], fp16, kind="ExternalInput")
    d_out = nc.dram_tensor("out", [S, NT_OUT, 64], f32, kind="ExternalOutput")
    d_probe = None
    if probe_layer >= 0:
        d_probe = nc.dram_tensor("probe", [2, 128, S, TH], f32r,
                                 kind="ExternalOutput")

    es = ExitStack()
    with tile.TileContext(nc) as tc, es:
        consts = es.enter_context(tc.tile_pool(name="consts", bufs=1))
        xbufs = es.enter_context(tc.tile_pool(name="xbufs", bufs=1))

        # ---- constants
        t_xc = consts.tile([5, S, T], fp16)
        nc.sync.dma_start(out=t_xc[:], in_=d_xc[:, :, :])
        t_wc0e = consts.tile([5, 256], fp16)
        nc.sync.dma_start(out=t_wc0e[:], in_=d_wc0e[:, :])
        t_gamma = consts.tile([128, 6], f32)
        nc.sync.dma_start(out=t_gamma[:], in_=d_gamma[:, :])
        t_beta = consts.tile([128, 6], f32)
        nc.sync.dma_start(out=t_beta[:], in_=d_beta[:, :])
        t_gind = consts.tile([128, 8], f32)
        nc.sync.dma_start(out=t_gind[:], in_=d_gind[:, :])
        t_gexp = consts.tile([8, 128], f32)
        nc.sync.dma_start(out=t_gexp[:], in_=d_gexp[:, :])
        t_eps = consts.tile([8, 1], f32)
        nc.vector.memset(t_eps[:], EPS)
        t_id128 = consts.tile([128, 128], fp16)
        nc.sync.dma_start(out=t_id128[:], in_=d_id128[:, :])
        # LSTM consts
        t_wihG = consts.tile([128, 4, 2, 64], fp16)
        nc.sync.dma_start(out=t_wihG[:], in_=d_wihG[:, :, :, :])
        t_whhG = consts.tile([65, 4, 64], fp16)
        nc.sync.dma_start(out=t_whhG[:], in_=d_whhG[:, :, :])
        t_ones64h = consts.tile([64, 64], fp16)
        nc.vector.memset(t_ones64h[:], 1.0)

        # ---- input activations (xbuf reused as interp output every layer)
        t_x = xbufs.tile([128, 2, S, TH], fp16)
        nc.sync.dma_start(out=t_x[:], in_=d_xab[:, :, :, :])

        def mm(out, lhsT, rhs, start, stop, dt=None, **kw):
            if dt is not None:
                lhsT = lhsT.bitcast(dt)
                rhs = rhs.bitcast(dt)
            nc.tensor.matmul(out=out, lhsT=lhsT, rhs=rhs, start=start,
                             stop=stop, **kw)

        # ================= conv + GN + interp layers =================
        with ExitStack() as ces:
            wpool = ces.enter_context(tc.tile_pool(name="wpool", bufs=1))
            hraw_p = ces.enter_context(tc.tile_pool(name="hraw", bufs=2))
            stats_p = ces.enter_context(tc.tile_pool(name="stats", bufs=2))
            small_p = ces.enter_context(tc.tile_pool(name="small", bufs=2))
            y_p = ces.enter_context(tc.tile_pool(name="ybuf", bufs=3))
            scr_p = ces.enter_context(tc.tile_pool(name="scr", bufs=3))
            sm_p = ces.enter_context(tc.tile_pool(name="smat", bufs=2))
            yt_p = ces.enter_context(tc.tile_pool(name="ytp", bufs=3))
            cpsum = ces.enter_context(
                tc.tile_pool(name="cpsum", bufs=2, space="PSUM"))
            stps = ces.enter_context(
                tc.tile_pool(name="stps", bufs=1, space="PSUM"))
            tpsum = ces.enter_context(
                tc.tile_pool(name="tpsum", bufs=2, space="PSUM"))
            sops = ces.enter_context(
                tc.tile_pool(name="sops", bufs=3, space="PSUM"))

            for l in range(3):
                t_wc = wpool.tile([128, 20, 128], fp16, tag="wconv")
                nc.sync.dma_start(
                    out=t_wc[:],
                    in_=bass.AP(tensor=d_wconv, offset=l * 20 * 128 * 128,
                                ap=[[128, 128], [128 * 128, 20], [1, 128]]))

                for grp in range(2):
                    sums = [stats_p.tile([128, SG], f32, tag=f"sums{h}", name=f"sums{h}")
                            for h in range(2)]
                    qs = [stats_p.tile([128, SG], f32, tag=f"qs{h}", name=f"qs{h}")
                          for h in range(2)]
                    hraw = [hraw_p.tile([128, SG, T], fp16, tag=f"hraw{h}", name=f"hraw{h}")
                            for h in range(2)]

                    # ---- phase 1: conv + fused stats
                    for pp in range(NPAIR):
                        pr = grp * NPAIR + pp
                        for h in range(2):
                            ps = cpsum.tile([128, 2, T], f32, tag="cps")
                            ops = []
                            for cc in range(2):
                                for k in range(5):
                                    ops.append((
                                        t_wc[:, (cc * 5 + k) * 2 + h, :],
                                        t_x[:, cc, 2 * pr:2 * pr + 2,
                                            k:k + T], None))
                            if l == 0:
                                ops.append((
                                    t_wc0e[:, h * 128:(h + 1) * 128],
                                    t_xc[:, 2 * pr:2 * pr + 2, :], None))
                            for j, (lh, rh, dt) in enumerate(ops):
                                mm(ps[:], lh, rh, j == 0, j == len(ops) - 1,
                                   dt=dt)
                            for i in range(2):
                                sl = pp * 2 + i
                                nc.scalar.activation(
                                    out=hraw[h][:, sl, :], in_=ps[:, i, :],
                                    func=AF.Identity,
                                    accum_out=sums[h][:, sl:sl + 1])
                                scr = scr_p.tile([128, T], fp16, tag="sq")
                                nc.vector.scalar_tensor_tensor(
                                    out=scr[:], in0=hraw[h][:, sl, :],
                                    scalar=1.0,
                                    in1=hraw[h][:, sl, :], op0=OP.mult,
                                    op1=OP.mult,
                                    accum_out=qs[h][:, sl:sl + 1])

                    # ---- phase 2: group stats -> A, B per half
                    AB = []
                    for h in range(2):
                        g1 = stps.tile([8, SG], f32, tag="gg")
                        mm(g1[:], t_gind[:], sums[h][:], True, True)
                        g2 = stps.tile([8, SG], f32, tag="gg")
                        mm(g2[:], t_gind[:], qs[h][:], True, True)
                        mean = small_p.tile([8, SG], f32, tag="mean")
                        nc.vector.tensor_scalar_mul(mean[:], g1[:],
                                                    1.0 / (GRP * T))
                        msq = small_p.tile([8, SG], f32, tag="msq")
                        nc.vector.tensor_tensor(out=msq[:], in0=mean[:],
                                                in1=mean[:], op=OP.mult)
                        var = small_p.tile([8, SG], f32, tag="var")
                        nc.vector.scalar_tensor_tensor(
                            out=var[:], in0=g2[:], scalar=1.0 / (GRP * T),
                            in1=msq[:], op0=OP.mult, op1=OP.subtract)
                        sd = small_p.tile([8, SG], f32, tag="sd")
                        nc.scalar.activation(out=sd[:], in_=var[:],
                                             func=AF.Sqrt,
                                             bias=t_eps[:, :1], scale=1.0)
                        rstd = small_p.tile([8, SG], f32, tag="rstd")
                        nc.vector.reciprocal(rstd[:], sd[:])
                        rp = stps.tile([128, SG], f32, tag="gg")
                        mm(rp[:], t_gexp[:], rstd[:], True, True)
                        mp = stps.tile([128, SG], f32, tag="gg")
                        mm(mp[:], t_gexp[:], mean[:], True, True)
                        At = small_p.tile([128, SG], f32, tag="A")
                        nc.vector.tensor_scalar_mul(
                            At[:], rp[:],
                            t_gamma[:, l * 2 + h:l * 2 + h + 1])
                        tmp = small_p.tile([128, SG], f32, tag="tmp")
                        nc.vector.tensor_tensor(out=tmp[:], in0=mp[:],
                                                in1=At[:], op=OP.mult)
                        Bt = small_p.tile([128, SG], f32, tag="B")
                        nc.vector.tensor_scalar(
                            out=Bt[:], in0=tmp[:], scalar1=-1.0,
                            scalar2=t_beta[:, l * 2 + h:l * 2 + h + 1],
                            op0=OP.mult, op1=OP.add)
                        AB.append((At, Bt))

                    # ---- phase 3: normalize+relu, transpose, interp matmul
                    for pp in range(NPAIR):
                        pr = grp * NPAIR + pp
                        s128 = sm_p.tile([128, 2, T], fp16, tag="s128")
                        nc.gpsimd.dma_start(
                            out=s128[:],
                            in_=d_wS[l, 2 * pr:2 * pr + 2, 0:128, :].rearrange(
                                "s t w -> t s w"))
                        s64 = sm_p.tile([64, 2, T], fp16, tag="s64")
                        nc.gpsimd.dma_start(
                            out=s64[:],
                            in_=d_wS[l, 2 * pr:2 * pr + 2, 128:192, :].rearrange(
                                "s t w -> t s w"))
                        for i in range(2):
                            sl = pp * 2 + i
                            sg_ = 2 * pr + i
                            yth = [yt_p.tile([128, 2, 128], fp16,
                                             tag=f"yth{h}", name=f"yth{h}")
                                   for h in range(2)]
                            for h in range(2):
                                At, Bt = AB[h]
                                ytmp = y_p.tile([128, T], fp16, tag="ytmp")
                                nc.vector.tensor_scalar(
                                    out=ytmp[:], in0=hraw[h][:, sl, :],
                                    scalar1=At[:, sl:sl + 1],
                                    scalar2=Bt[:, sl:sl + 1],
                                    op0=OP.mult, op1=OP.add)
                                yb = y_p.tile([128, T], fp16, tag="yb")
                                nc.vector.tensor_scalar_max(
                                    yb[:], ytmp[:], 0.0)
                                ptp = tpsum.tile([128, 2, 128], fp16,
                                                 tag="tp", name="ptp")
                                nc.tensor.transpose(
                                    out=ptp[:, 0, :], in_=yb[:, 0:128],
                                    identity=t_id128[:])
                                nc.tensor.transpose(
                                    out=ptp[0:64, 1, :], in_=yb[:, 128:192],
                                    identity=t_id128[:])
                                nc.vector.tensor_copy(
                                    out=yth[h][:], in_=ptp[:, :, :])
                            sout = sops.tile([128, 2, T], f32, tag="so",
                                             name="sout")
                            for ch in range(2):
                                mm(sout[:, ch, :], yth[ch][:, 0, :],
                                   s128[:, i, :], True, False)
                                mm(sout[:, ch, :], yth[ch][0:64, 1, :],
                                   s64[:, i, :], False, True)
                            nc.vector.tensor_copy(
                                out=t_x[:, :, sg_, 2:194],
                                in_=sout[:, :, :])

                if probe_layer == l:
                    for h in range(2):
                        nc.gpsimd.dma_start(out=d_probe[h, :, :, :],
                                            in_=t_x[:, h, :, :])

        # ======================= biLSTM (gate-major) =======================
        # state tiles [64 part=(dir,unit), 64 cols=sample]; gates in PSUM
        # blocks [64, LBLK, 4(gate i,f,g,o), 64], one PSUM bank per block.
        lsb = es.enter_context(tc.tile_pool(name="lstm_sbuf", bufs=1))
        t_SIG = lsb.tile([64, 192], fp16, name="sig")    # sig(i,f,2g)
        t_SIGO = lsb.tile([64, 64], fp16, name="sigo")   # sig(o)
        t_TG = lsb.tile([64, 128], fp16, name="tg")      # [tanh g | c]
        t_P = lsb.tile([64, 128], fp16, name="pp")       # [si*tg | sf*c]
        t_TC = lsb.tile([64, 64], fp16, name="tc")       # tanh(c)
        t_HT = lsb.tile([65, 64], fp16, name="ht")       # h; row 64 = ones
        t_OUT = lsb.tile([64, NT_OUT, 64], f32, name="outt")
        nc.vector.memset(t_TG[:, 64:128], 0.0)
        nc.vector.memset(t_HT[0:64, :], 0.0)
        nc.vector.memset(t_HT[64:65, :], 1.0)

        with tc.tile_pool(name="lpsum", bufs=2, space="PSUM") as lpsum, \
             tc.tile_pool(name="ltp", bufs=2, space="PSUM") as ltp:

            def xw_half(blk, half, G=None):
                """xw preacts for gates [2*half, 2*half+2) of one block.
                start=True clears the full bank row for the partitions the
                matmul writes -> one start per direction, on its first
                matmul (half 0)."""
                t0 = blk * LBLK
                if G is None:
                    G = lpsum.tile([64, LBLK, 4, 64], f32, tag="xw",
                                   name="xw")
                for g4 in range(2 * half, 2 * half + 2):
                    for cc in range(2):
                        # dir f reads time t0..t0+LBLK-1 (+2 halo offset)
                        rf = t_x[:, cc, :, 2 + t0:2 + t0 + LBLK].rearrange(
                            "c s t -> c t s")
                        mm(G[0:32, :, g4, :], t_wihG[:, g4, cc, 0:32], rf,
                           g4 == 2 * half == 0 and cc == 0, False)
                        # dir b reads time 193-t0 downward (negative stride)
                        base = t_x[:, cc, :, 0:LBLK].rearrange("c s t -> c t s")
                        rb = bass.AP(tensor=base.tensor,
                                     offset=base.offset + (193 - t0),
                                     ap=[base.ap[0], [-1, LBLK], base.ap[2]])
                        mm(G[32:64, :, g4, :], t_wihG[:, g4, cc, 32:64], rb,
                           g4 == 2 * half == 0 and cc == 0, False)
                return G

            xwp = [xw_half(0, 1, xw_half(0, 0)), None]
            t_OUTT = lsb.tile([64, NT_OUT, 64], fp16, name="outth")
            if probe_layer == 4:
                t_gdbg = lsb.tile([64, LBLK, 4, 64], f32, name="gdbg")
                nc.vector.tensor_copy(out=t_gdbg[:], in_=xwp[0][:, :, :, :])
                nc.gpsimd.dma_start(out=d_probe[0, 0:64, 0:LBLK * 4, 0:64],
                                    in_=t_gdbg[:, :, :, :])

            for g in range(T):
                blk, j = g // LBLK, g % LBLK
                G = xwp[blk % 2]
                for g4 in range(4):
                    mm(G[:, j, g4, :], t_whhG[:, g4, :], t_HT[:],
                       False, True, skip_group_check=True)
                nc.scalar.activation(
                    out=t_SIG[:],
                    in_=G[:, j, 0:3, :].rearrange("p g s -> p (g s)"),
                    func=AF.Sigmoid)
                nc.scalar.activation(out=t_SIGO[:], in_=G[:, j, 3, :],
                                     func=AF.Sigmoid)
                # tanh(g) = 2*sig(2g) - 1 into TG[:,0:64]
                nc.vector.scalar_tensor_tensor(
                    out=t_TG[:, 0:64], in0=t_SIG[:, 128:192], scalar=2.0,
                    in1=t_ones64h[:], op0=OP.mult, op1=OP.subtract)
                # [si*tg | sf*c]
                nc.vector.tensor_tensor(out=t_P[:], in0=t_SIG[:, 0:128],
                                        in1=t_TG[:, 0:128], op=OP.mult)
                # c = si*tg + sf*c
                nc.vector.tensor_tensor(out=t_TG[:, 64:128],
                                        in0=t_P[:, 0:64],
                                        in1=t_P[:, 64:128], op=OP.add)
                nc.scalar.activation(out=t_TC[:], in_=t_TG[:, 64:128],
                                     func=AF.Tanh)
                nc.vector.tensor_tensor(out=t_HT[0:64, :], in0=t_SIGO[:],
                                        in1=t_TC[:], op=OP.mult)
                if g % FREQ == FREQ - 1:
                    nc.vector.tensor_copy(
                        out=t_OUTT[0:32, g // FREQ, :], in_=t_HT[0:32, :])
                    nc.vector.tensor_copy(
                        out=t_OUTT[32:64, (T - 1 - g) // FREQ, :],
                        in_=t_HT[32:64, :])
                if j == 0 and blk + 1 < NBLK:
                    xwp[(blk + 1) % 2] = xw_half(blk + 1, 0)
                if j == 1 and blk + 1 < NBLK:
                    xw_half(blk + 1, 1, xwp[(blk + 1) % 2])
                if probe_layer == 5 and g == 0:
                    nc.gpsimd.dma_start(out=d_probe[0, 0:64, 0, 0:192],
                                        in_=t_SIG[:, :])
                    nc.gpsimd.dma_start(out=d_probe[0, 0:64, 1, 0:64],
                                        in_=t_SIGO[:, :])
                    nc.gpsimd.dma_start(out=d_probe[0, 0:64, 2, 0:128],
                                        in_=t_TG[:, :])
                    nc.gpsimd.dma_start(out=d_probe[0, 0:64, 3, 0:64],
                                        in_=t_TC[:, :])
                    nc.gpsimd.dma_start(out=d_probe[0, 0:64, 4, 0:64],
                                        in_=t_HT[:, :])
            # post-loop: transpose the collected h tiles to [sample, du]
            for k in range(NT_OUT):
                pht = ltp.tile([64, 64], fp16, tag="pht", name="pht")
                nc.tensor.transpose(out=pht[:], in_=t_OUTT[:, k, :],
                                    identity=t_id128[0:64, 0:64])
                nc.vector.tensor_copy(out=t_OUT[:, k, :], in_=pht[:])

        nc.sync.dma_start(out=d_out[:, :, :], in_=t_OUT[:])

    nc.compile()
    return nc


def _get_nc(probe_layer=-1):
    key = ("nc", probe_layer)
    if key not in _cache:
        _cache[key] = _build(probe_layer)
    return _cache[key]


def run_on_cores(inputs, probe_layer=-1, trace=False):
    """Build (cached), run on 8 cores; returns (results, BassKernelResults)."""
    from concourse.bass_utils import run_bass_kernel_spmd

    nc = _get_nc(probe_layer)
    in_maps = _prep_host(inputs)
    last_exc = None
    for _ in range(3):
        try:
            res = run_bass_kernel_spmd(nc, in_maps,
                                       core_ids=list(range(N_CORES)),
                                       trace=trace)
            return res
        except Exception as e:  # transient NRT errors happen; retry
            last_exc = e
    raise last_exc


def assemble_output(res):
    out = np.zeros((B, NT_OUT, 64), np.float32)
    for core in range(N_CORES):
        s0 = core * S
        out[s0:s0 + S] = res.results[core]["out"]
    return out


def kernel(**inputs):
    res = run_on_cores(inputs)
    return assemble_output(res)



# revision 9
# speedup vs baseline: 1.0216x; 1.0216x over previous
"""Trainium2 Bass kernel for nn_Encoder_6 (conv+GN+InterpLnr x3 -> biLSTM).

Self-contained: host-side prep (sharding, interp gather tables, weight
repacking) + Bass/Tile device kernel + output gather.

Data-parallel over 8 NeuronCores: 64 samples per core.

Device dataflow per core (all samples resident on-chip after one load):
  - activations live in [channel(partition), sample, time] layout
  - conv1d = 10-11 accumulating matmuls per sample-pair (taps x cin-chunks),
    PSUM [128, 2x192]
  - GroupNorm stats fused into PSUM evacuation (ACT copy+accum -> sums,
    DVE square+accum -> sumsq), group reduce + expand via tiny matmuls
  - normalize+ReLU = single ACT op (per-partition scale/bias)
  - InterpLnr = banded-matrix matmul per sample (layer 2 also emits a
    time-reversed copy for the backward LSTM direction)
  - biLSTM in gate-major layout: state tiles are [64=(dir,unit), sample];
    per step: 4 tiny recurrence matmuls into the PSUM gate block, one
    sigmoid ACT over (i,f,2g) [tanh(g)=2*sig(2g)-1, the 2x folded into
    host weights], one sigmoid ACT for o, 3 DVE ops for the cell update,
    tanh ACT, 1 DVE for h.  No per-step transposes.
"""
import sys
from contextlib import ExitStack

sys.path.insert(0, "/opt/trn_rl_repo")

import numpy as np
import ml_dtypes

B = 512
N_CORES = 8
S = B // N_CORES          # samples per core
DIM_PIT = 257
C = 256                   # conv channels
T = 192                   # padded time
TH = 196                  # time with halo (2 each side)
GRP = 16                  # channels per group
DIM_NECK = 32
FREQ = 8
NT_OUT = 24               # output timesteps per direction
MIN_LEN_SEG = 19
MAX_NUM_SEG = 7
W64 = 64                  # 2*MAX_LEN_SEG
EPS = 1e-5
SG = 32                   # samples per stats group (2 groups per core)
NPAIR = 16                # sample pairs per stats group
LBLK = 2                  # LSTM timesteps per PSUM block (block = 1 PSUM bank)
NBLK = T // LBLK

_cache = {}


# ---------------------------------------------------------------- host prep

def _interp_tables(scales_u, len_seg_raw, n):
    """Gather idx/w1/w2 per sample for one interp layer (numpy, exact)."""
    scales = scales_u.astype(np.float32) + np.float32(0.5)
    j = np.arange(W64, dtype=np.float32)
    idx_scaled = j[None, :] / scales[:, None]
    idx_fl = np.floor(idx_scaled)
    lam = idx_scaled - idx_fl
    len_seg = (len_seg_raw + MIN_LEN_SEG).astype(np.float32)[:, None]
    idx_mask = idx_fl < (len_seg - 1.0)
    ls = (len_seg_raw + MIN_LEN_SEG).reshape(n, MAX_NUM_SEG)
    offset = np.cumsum(ls, axis=-1)
    offset = np.pad(offset[:, :-1], ((0, 0), (1, 0))).reshape(-1, 1)
    idx_org = idx_fl + offset.astype(np.float32)
    mask = (idx_mask & (idx_org < (T - 1))).reshape(n, MAX_NUM_SEG * W64)
    idx_b = np.clip(idx_org.reshape(n, -1).astype(np.int32), 0, T - 2)
    lam_b = lam.reshape(n, -1)
    idx = np.zeros((n, T), np.int32)
    w1 = np.zeros((n, T), np.float32)
    w2 = np.zeros((n, T), np.float32)
    for b in range(n):
        js = np.nonzero(mask[b])[0][:T]
        k = len(js)
        idx[b, :k] = idx_b[b, js]
        w1[b, :k] = 1.0 - lam_b[b, js]
        w2[b, :k] = lam_b[b, js]
    return idx, w1, w2


def _prep_host(inputs):
    """Build per-core input dicts. Returns list of 8 dicts."""
    x = np.asarray(inputs["x"], np.float32)
    scales = np.asarray(inputs["scales"], np.float32)
    lsr = np.asarray(inputs["len_seg_raw"], np.int32)

    # conv weights as lhsT tiles [l, chunk, tap, half, cin128, cout128]
    wconv = np.zeros((3, 2, 5, 2, 128, 128), np.float32)
    for l in range(3):
        w = np.asarray(inputs[f"conv{l}_w"], np.float32)  # [256, cin, 5]
        for cc in range(2):
            for k in range(5):
                for h in range(2):
                    wconv[l, cc, k, h] = w[h * 128:(h + 1) * 128,
                                           cc * 128:(cc + 1) * 128, k].T
    wconv = np.ascontiguousarray(wconv.astype(np.float16))
    # conv0 channel 256 as [5, 256] lhsT (k=tap)
    w0 = np.asarray(inputs["conv0_w"], np.float32)
    wc0e = np.ascontiguousarray(w0[:, 256, :].T.astype(np.float16))  # [5, 256]

    conv_bias = [np.asarray(inputs[f"conv{l}_b"], np.float32) for l in range(3)]
    assert all(np.abs(b).max() == 0.0 for b in conv_bias), \
        "nonzero conv bias not implemented in device kernel"

    gamma_t = np.stack([np.asarray(inputs[f"gn{l}_g"], np.float32).reshape(2, 128)
                        for l in range(3)])          # [3, 2, 128]
    beta_t = np.stack([np.asarray(inputs[f"gn{l}_b"], np.float32).reshape(2, 128)
                       for l in range(3)])
    gamma_t = np.ascontiguousarray(gamma_t.transpose(2, 0, 1).reshape(128, 6))
    beta_t = np.ascontiguousarray(beta_t.transpose(2, 0, 1).reshape(128, 6))

    gind = np.zeros((128, 8), np.float32)
    for c in range(128):
        gind[c, c // 16] = 1.0
    gexp = np.ascontiguousarray(gind.T)               # [8, 128]

    # interp tables, all samples
    idx_all, w1_all, w2_all = [], [], []
    for l in range(3):
        idx, w1, w2 = _interp_tables(scales[l], lsr[l], B)
        idx_all.append(idx)
        w1_all.append(w1)
        w2_all.append(w2)

    # LSTM weights, gate-major layout. gate order i,f,g,o; g-gate scaled
    # by 2 (tanh(g) = 2*sigmoid(2g) - 1 on device).
    #  wihG [128 cin, 4 gate, 2 cc, 64 (d,u)]  lhsT of xw matmuls
    #  whhG [64 (d,u'), 4 gate, 64 (d,u)]      lhsT of recurrence matmuls
    #  biasG [4 gate, 64 (d,u)]                lhsT of rank-1 bias matmuls
    H = DIM_NECK
    wihG = np.zeros((128, 4, 2, 64), np.float32)
    whhG = np.zeros((65, 4, 64), np.float32)   # row 64 = bias (ones in rhs)
    for d, nm in enumerate(["f", "b"]):
        wi = np.asarray(inputs[f"w_ih_{nm}"], np.float32)   # [128, 256]
        wh = np.asarray(inputs[f"w_hh_{nm}"], np.float32)   # [128, 32]
        bb = (np.asarray(inputs[f"b_ih_{nm}"], np.float32)
              + np.asarray(inputs[f"b_hh_{nm}"], np.float32))
        for g in range(4):
            sc = 2.0 if g == 2 else 1.0
            for cc in range(2):
                wihG[:, g, cc, d * H:(d + 1) * H] = \
                    sc * wi[g * H:(g + 1) * H, cc * 128:(cc + 1) * 128].T
            whhG[d * H:(d + 1) * H, g, d * H:(d + 1) * H] = \
                sc * wh[g * H:(g + 1) * H, :].T
            whhG[64, g, d * H:(d + 1) * H] = sc * bb[g * H:(g + 1) * H]
    wihG = np.ascontiguousarray(wihG.astype(np.float16))
    whhG = np.ascontiguousarray(whhG.astype(np.float16))

    in_maps = []
    for core in range(N_CORES):
        s0 = core * S
        xs = x[s0:s0 + S]                              # [S, 257, 192]
        xt = xs.transpose(1, 0, 2)                     # [257, S, 192]
        xab = np.zeros((128, 2, S, TH), np.float32)
        xab[:, 0, :, 2:194] = xt[:128]
        xab[:, 1, :, 2:194] = xt[128:256]
        xc = np.zeros((5, S, T), np.float32)
        x256 = xt[256]                                 # [S, 192]
        for k in range(5):
            sh = k - 2
            lo, hi = max(0, -sh), min(T, T - sh)
            xc[k, :, lo:hi] = x256[:, lo + sh:hi + sh]

        # banded interp matrices S[t_in, t_out] per (layer, sample), fp16
        wS = np.zeros((3, S, T, T), np.float16)
        bi = np.arange(S)[:, None]
        pj = np.arange(T)[None, :]
        for l in range(3):
            idx = idx_all[l][s0:s0 + S]
            Sm = np.zeros((S, T, T), np.float32)
            Sm[bi, idx, pj] = w1_all[l][s0:s0 + S]
            Sm[bi, idx + 1, pj] += w2_all[l][s0:s0 + S]
            wS[l] = Sm.astype(np.float16)

        in_maps.append({
            "xab": np.ascontiguousarray(xab.astype(np.float16)),
            "xc": np.ascontiguousarray(xc.astype(np.float16)),
            "wconv": wconv,
            "wc0e": wc0e,
            "gamma_t": gamma_t,
            "beta_t": beta_t,
            "gind": gind,
            "gexp": gexp,
            "wS": np.ascontiguousarray(wS),
            "id128": np.eye(128, dtype=np.float16),
            "wihG": wihG,
            "whhG": whhG,
        })
    return in_maps


# ------------------------------------------------------------- device build

def _build(probe_layer=-1):
    """Build the Bacc module. probe_layer >= 0 adds a probe output of XBUF
    after that layer's interp (for debugging)."""
    import concourse.bass as bass
    import concourse.tile as tile
    from concourse import bacc, mybir
    from concourse.masks import make_identity

    f32 = mybir.dt.float32
    f32r = mybir.dt.float32r
    bf16 = mybir.dt.bfloat16
    fp16 = mybir.dt.float16
    AF = mybir.ActivationFunctionType
    OP = mybir.AluOpType

    nc = bacc.Bacc("TRN2", target_bir_lowering=False, debug=False,
                   enable_asserts=False, num_devices=N_CORES)

    # DRAM tensors
    d_xab = nc.dram_tensor("xab", [128, 2, S, TH], fp16, kind="ExternalInput")
    d_xc = nc.dram_tensor("xc", [5, S, T], fp16, kind="ExternalInput")
    d_wconv = nc.dram_tensor("wconv", [3, 2, 5, 2, 128, 128], fp16,
                             kind="ExternalInput")
    d_wc0e = nc.dram_tensor("wc0e", [5, 256], fp16, kind="ExternalInput")
    d_gamma = nc.dram_tensor("gamma_t", [128, 6], f32, kind="ExternalInput")
    d_beta = nc.dram_tensor("beta_t", [128, 6], f32, kind="ExternalInput")
    d_gind = nc.dram_tensor("gind", [128, 8], f32, kind="ExternalInput")
    d_gexp = nc.dram_tensor("gexp", [8, 128], f32, kind="ExternalInput")
    d_wS = nc.dram_tensor("wS", [3, S, T, T], fp16, kind="ExternalInput")
    d_id128 = nc.dram_tensor("id128", [128, 128], fp16, kind="ExternalInput")
    d_wihG = nc.dram_tensor("wihG", [128, 4, 2, 64], fp16,
                            kind="ExternalInput")
    d_whhG = nc.dram_tensor("whhG", [65, 4, 64], fp16, kind="ExternalInput")
    d_out = nc.dram_tensor("out", [S, NT_OUT, 64], f32, kind="ExternalOutput")
    d_probe = None
    if probe_layer >= 0:
        d_probe = nc.dram_tensor("probe", [2, 128, S, TH], f32r,
                                 kind="ExternalOutput")

    es = ExitStack()
    with tile.TileContext(nc) as tc, es:
        consts = es.enter_context(tc.tile_pool(name="consts", bufs=1))
        xbufs = es.enter_context(tc.tile_pool(name="xbufs", bufs=1))

        # ---- constants
        t_xc = consts.tile([5, S, T], fp16)
        nc.sync.dma_start(out=t_xc[:], in_=d_xc[:, :, :])
        t_wc0e = consts.tile([5, 256], fp16)
        nc.sync.dma_start(out=t_wc0e[:], in_=d_wc0e[:, :])
        t_gamma = consts.tile([128, 6], f32)
        nc.sync.dma_start(out=t_gamma[:], in_=d_gamma[:, :])
        t_beta = consts.tile([128, 6], f32)
        nc.sync.dma_start(out=t_beta[:], in_=d_beta[:, :])
        t_gind = consts.tile([128, 8], f32)
        nc.sync.dma_start(out=t_gind[:], in_=d_gind[:, :])
        t_gexp = consts.tile([8, 128], f32)
        nc.sync.dma_start(out=t_gexp[:], in_=d_gexp[:, :])
        t_eps = consts.tile([8, 1], f32)
        nc.vector.memset(t_eps[:], EPS)
        t_id128 = consts.tile([128, 128], fp16)
        nc.sync.dma_start(out=t_id128[:], in_=d_id128[:, :])
        # LSTM consts
        t_wihG = consts.tile([128, 4, 2, 64], fp16)
        nc.sync.dma_start(out=t_wihG[:], in_=d_wihG[:, :, :, :])
        t_whhG = consts.tile([65, 4, 64], fp16)
        nc.sync.dma_start(out=t_whhG[:], in_=d_whhG[:, :, :])
        t_ones64h = consts.tile([64, 64], fp16)
        nc.vector.memset(t_ones64h[:], 1.0)

        # ---- input activations (xbuf reused as interp output every layer)
        t_x = xbufs.tile([128, 2, S, TH], fp16)
        nc.sync.dma_start(out=t_x[:], in_=d_xab[:, :, :, :])
        # layer-2 interp output in [c, cc, t, s] layout: LSTM xw matmuls read
        # contiguous (t, s) runs instead of stride-196 sample-major slices
        t_xT = xbufs.tile([128, 2, T, S], fp16)

        def mm(out, lhsT, rhs, start, stop, dt=None, **kw):
            if dt is not None:
                lhsT = lhsT.bitcast(dt)
                rhs = rhs.bitcast(dt)
            nc.tensor.matmul(out=out, lhsT=lhsT, rhs=rhs, start=start,
                             stop=stop, **kw)

        # ================= conv + GN + interp layers =================
        with ExitStack() as ces:
            wpool = ces.enter_context(tc.tile_pool(name="wpool", bufs=1))
            hraw_p = ces.enter_context(tc.tile_pool(name="hraw", bufs=2))
            stats_p = ces.enter_context(tc.tile_pool(name="stats", bufs=2))
            small_p = ces.enter_context(tc.tile_pool(name="small", bufs=2))
            y_p = ces.enter_context(tc.tile_pool(name="ybuf", bufs=3))
            scr_p = ces.enter_context(tc.tile_pool(name="scr", bufs=3))
            sm_p = ces.enter_context(tc.tile_pool(name="smat", bufs=2))
            yt_p = ces.enter_context(tc.tile_pool(name="ytp", bufs=3))
            cpsum = ces.enter_context(
                tc.tile_pool(name="cpsum", bufs=2, space="PSUM"))
            stps = ces.enter_context(
                tc.tile_pool(name="stps", bufs=1, space="PSUM"))
            tpsum = ces.enter_context(
                tc.tile_pool(name="tpsum", bufs=2, space="PSUM"))
            sops = ces.enter_context(
                tc.tile_pool(name="sops", bufs=3, space="PSUM"))

            for l in range(3):
                t_wc = wpool.tile([128, 20, 128], fp16, tag="wconv")
                nc.sync.dma_start(
                    out=t_wc[:],
                    in_=bass.AP(tensor=d_wconv, offset=l * 20 * 128 * 128,
                                ap=[[128, 128], [128 * 128, 20], [1, 128]]))

                for grp in range(2):
                    sums = [stats_p.tile([128, SG], f32, tag=f"sums{h}", name=f"sums{h}")
                            for h in range(2)]
                    qs = [stats_p.tile([128, SG], f32, tag=f"qs{h}", name=f"qs{h}")
                          for h in range(2)]
                    hraw = [hraw_p.tile([128, SG, T], fp16, tag=f"hraw{h}", name=f"hraw{h}")
                            for h in range(2)]

                    # ---- phase 1: conv + fused stats
                    for pp in range(NPAIR):
                        pr = grp * NPAIR + pp
                        for h in range(2):
                            ps = cpsum.tile([128, 2, T], f32, tag="cps")
                            ops = []
                            for cc in range(2):
                                for k in range(5):
                                    ops.append((
                                        t_wc[:, (cc * 5 + k) * 2 + h, :],
                                        t_x[:, cc, 2 * pr:2 * pr + 2,
                                            k:k + T], None))
                            if l == 0:
                                ops.append((
                                    t_wc0e[:, h * 128:(h + 1) * 128],
                                    t_xc[:, 2 * pr:2 * pr + 2, :], None))
                            for j, (lh, rh, dt) in enumerate(ops):
                                mm(ps[:], lh, rh, j == 0, j == len(ops) - 1,
                                   dt=dt)
                            for i in range(2):
                                sl = pp * 2 + i
                                nc.scalar.activation(
                                    out=hraw[h][:, sl, :], in_=ps[:, i, :],
                                    func=AF.Identity,
                                    accum_out=sums[h][:, sl:sl + 1])
                                scr = scr_p.tile([128, T], fp16, tag="sq")
                                nc.vector.scalar_tensor_tensor(
                                    out=scr[:], in0=hraw[h][:, sl, :],
                                    scalar=1.0,
                                    in1=hraw[h][:, sl, :], op0=OP.mult,
                                    op1=OP.mult,
                                    accum_out=qs[h][:, sl:sl + 1])

                    # ---- phase 2: group stats -> A, B per half
                    AB = []
                    for h in range(2):
                        g1 = stps.tile([8, SG], f32, tag="gg")
                        mm(g1[:], t_gind[:], sums[h][:], True, True)
                        g2 = stps.tile([8, SG], f32, tag="gg")
                        mm(g2[:], t_gind[:], qs[h][:], True, True)
                        mean = small_p.tile([8, SG], f32, tag="mean")
                        nc.vector.tensor_scalar_mul(mean[:], g1[:],
                                                    1.0 / (GRP * T))
                        msq = small_p.tile([8, SG], f32, tag="msq")
                        nc.vector.tensor_tensor(out=msq[:], in0=mean[:],
                                                in1=mean[:], op=OP.mult)
                        var = small_p.tile([8, SG], f32, tag="var")
                        nc.vector.scalar_tensor_tensor(
                            out=var[:], in0=g2[:], scalar=1.0 / (GRP * T),
                            in1=msq[:], op0=OP.mult, op1=OP.subtract)
                        sd = small_p.tile([8, SG], f32, tag="sd")
                        nc.scalar.activation(out=sd[:], in_=var[:],
                                             func=AF.Sqrt,
                                             bias=t_eps[:, :1], scale=1.0)
                        rstd = small_p.tile([8, SG], f32, tag="rstd")
                        nc.vector.reciprocal(rstd[:], sd[:])
                        rp = stps.tile([128, SG], f32, tag="gg")
                        mm(rp[:], t_gexp[:], rstd[:], True, True)
                        mp = stps.tile([128, SG], f32, tag="gg")
                        mm(mp[:], t_gexp[:], mean[:], True, True)
                        At = small_p.tile([128, SG], f32, tag="A")
                        nc.vector.tensor_scalar_mul(
                            At[:], rp[:],
                            t_gamma[:, l * 2 + h:l * 2 + h + 1])
                        tmp = small_p.tile([128, SG], f32, tag="tmp")
                        nc.vector.tensor_tensor(out=tmp[:], in0=mp[:],
                                                in1=At[:], op=OP.mult)
                        Bt = small_p.tile([128, SG], f32, tag="B")
                        nc.vector.tensor_scalar(
                            out=Bt[:], in0=tmp[:], scalar1=-1.0,
                            scalar2=t_beta[:, l * 2 + h:l * 2 + h + 1],
                            op0=OP.mult, op1=OP.add)
                        AB.append((At, Bt))

                    # ---- phase 3: normalize+relu, transpose, interp matmul
                    for pp in range(NPAIR):
                        pr = grp * NPAIR + pp
                        s128 = sm_p.tile([128, 2, T], fp16, tag="s128")
                        nc.gpsimd.dma_start(
                            out=s128[:],
                            in_=d_wS[l, 2 * pr:2 * pr + 2, 0:128, :].rearrange(
                                "s t w -> t s w"))
                        s64 = sm_p.tile([64, 2, T], fp16, tag="s64")
                        nc.gpsimd.dma_start(
                            out=s64[:],
                            in_=d_wS[l, 2 * pr:2 * pr + 2, 128:192, :].rearrange(
                                "s t w -> t s w"))
                        for i in range(2):
                            sl = pp * 2 + i
                            sg_ = 2 * pr + i
                            yth = [yt_p.tile([128, 2, 128], fp16,
                                             tag=f"yth{h}", name=f"yth{h}")
                                   for h in range(2)]
                            for h in range(2):
                                At, Bt = AB[h]
                                # norm + relu fused into one ACT op:
                                # relu(A*h + B) with per-partition scale/bias
                                yb = y_p.tile([128, T], fp16, tag="yb")
                                nc.scalar.activation(
                                    out=yb[:], in_=hraw[h][:, sl, :],
                                    func=AF.Relu,
                                    bias=Bt[:, sl:sl + 1],
                                    scale=At[:, sl:sl + 1])
                                ptp = tpsum.tile([128, 2, 128], fp16,
                                                 tag="tp", name="ptp")
                                nc.tensor.transpose(
                                    out=ptp[:, 0, :], in_=yb[:, 0:128],
                                    identity=t_id128[:])
                                nc.tensor.transpose(
                                    out=ptp[0:64, 1, :], in_=yb[:, 128:192],
                                    identity=t_id128[:])
                                nc.vector.tensor_copy(
                                    out=yth[h][:], in_=ptp[:, :, :])
                            sout = sops.tile([128, 2, T], f32, tag="so",
                                             name="sout")
                            for ch in range(2):
                                mm(sout[:, ch, :], yth[ch][:, 0, :],
                                   s128[:, i, :], True, False)
                                mm(sout[:, ch, :], yth[ch][0:64, 1, :],
                                   s64[:, i, :], False, True)
                            if l < 2:
                                nc.vector.tensor_copy(
                                    out=t_x[:, :, sg_, 2:194],
                                    in_=sout[:, :, :])
                            else:
                                nc.vector.tensor_copy(
                                    out=t_xT[:, :, :, sg_],
                                    in_=sout[:, :, :])

                if probe_layer == l:
                    for h in range(2):
                        nc.gpsimd.dma_start(out=d_probe[h, :, :, :],
                                            in_=t_x[:, h, :, :])

        # ======================= biLSTM (gate-major) =======================
        # state tiles [64 part=(dir,unit), 64 cols=sample]; gates in PSUM
        # blocks [64, LBLK, 4(gate i,f,g,o), 64], one PSUM bank per block.
        lsb = es.enter_context(tc.tile_pool(name="lstm_sbuf", bufs=1))
        t_SIG = lsb.tile([64, 4, 64], fp16, name="sig")  # sig(i,f,2g,o)
        t_TG = lsb.tile([64, 128], fp16, name="tg")      # [tanh g | c]
        t_P = lsb.tile([64, 128], fp16, name="pp")       # [si*tg | sf*c]
        t_TC = lsb.tile([64, 64], fp16, name="tc")       # tanh(c)
        t_HT = lsb.tile([65, 64], fp16, name="ht")       # h; row 64 = ones
        t_OUT = lsb.tile([64, NT_OUT, 64], f32, name="outt")
        nc.vector.memset(t_TG[:, 64:128], 0.0)
        nc.vector.memset(t_HT[0:64, :], 0.0)
        nc.vector.memset(t_HT[64:65, :], 1.0)

        with tc.tile_pool(name="lpsum", bufs=2, space="PSUM") as lpsum, \
             tc.tile_pool(name="ltp", bufs=2, space="PSUM") as ltp:

            def xw_half(blk, half, G=None):
                """xw preacts for gates [2*half, 2*half+2) of one block.
                start=True clears the full bank row for the partitions the
                matmul writes -> one start per direction, on its first
                matmul (half 0)."""
                t0 = blk * LBLK
                if G is None:
                    G = lpsum.tile([64, LBLK, 4, 64], f32, tag="xw",
                                   name="xw")
                for g4 in range(2 * half, 2 * half + 2):
                    for cc in range(2):
                        # dir f reads time t0..t0+LBLK-1 (contiguous (t, s))
                        rf = t_xT[:, cc, t0:t0 + LBLK, :]
                        mm(G[0:32, :, g4, :], t_wihG[:, g4, cc, 0:32], rf,
                           g4 == 2 * half == 0 and cc == 0, False)
                        # dir b reads time 191-t0 downward (negative t stride)
                        base = t_xT[:, cc, 0:LBLK, :]
                        rb = bass.AP(tensor=base.tensor,
                                     offset=base.offset + (191 - t0) * S,
                                     ap=[base.ap[0], [-S, LBLK], [1, S]])
                        mm(G[32:64, :, g4, :], t_wihG[:, g4, cc, 32:64], rb,
                           g4 == 2 * half == 0 and cc == 0, False)
                return G

            xwp = [xw_half(0, 1, xw_half(0, 0)), None]
            t_OUTT = lsb.tile([64, NT_OUT, 64], fp16, name="outth")
            if probe_layer == 4:
                t_gdbg = lsb.tile([64, LBLK, 4, 64], f32, name="gdbg")
                nc.vector.tensor_copy(out=t_gdbg[:], in_=xwp[0][:, :, :, :])
                nc.gpsimd.dma_start(out=d_probe[0, 0:64, 0:LBLK * 4, 0:64],
                                    in_=t_gdbg[:, :, :, :])

            for g in range(T):
                blk, j = g // LBLK, g % LBLK
                G = xwp[blk % 2]
                for g4 in range(4):
                    mm(G[:, j, g4, :], t_whhG[:, g4, :], t_HT[:],
                       False, True, skip_group_check=True)
                nc.scalar.activation(
                    out=t_SIG[:], in_=G[:, j, :, :],
                    func=AF.Sigmoid)
                # tanh(g) = 2*sig(2g) - 1 into TG[:,0:64]
                nc.vector.scalar_tensor_tensor(
                    out=t_TG[:, 0:64], in0=t_SIG[:, 2, :], scalar=2.0,
                    in1=t_ones64h[:], op0=OP.mult, op1=OP.subtract)
                # [si*tg | sf*c]
                nc.vector.tensor_tensor(
                    out=t_P[:],
                    in0=t_SIG[:, 0:2, :].rearrange("p g s -> p (g s)"),
                    in1=t_TG[:, 0:128], op=OP.mult)
                # c = si*tg + sf*c
                nc.vector.tensor_tensor(out=t_TG[:, 64:128],
                                        in0=t_P[:, 0:64],
                                        in1=t_P[:, 64:128], op=OP.add)
                nc.scalar.activation(out=t_TC[:], in_=t_TG[:, 64:128],
                                     func=AF.Tanh)
                nc.vector.tensor_tensor(out=t_HT[0:64, :],
                                        in0=t_SIG[:, 3, :],
                                        in1=t_TC[:], op=OP.mult)
                if g % FREQ == FREQ - 1:
                    nc.vector.tensor_copy(
                        out=t_OUTT[0:32, g // FREQ, :], in_=t_HT[0:32, :])
                    nc.vector.tensor_copy(
                        out=t_OUTT[32:64, (T - 1 - g) // FREQ, :],
                        in_=t_HT[32:64, :])
                if j == 0 and blk + 1 < NBLK:
                    xwp[(blk + 1) % 2] = xw_half(blk + 1, 0)
                if j == 1 and blk + 1 < NBLK:
                    xw_half(blk + 1, 1, xwp[(blk + 1) % 2])
                if probe_layer == 5 and g == 0:
                    nc.gpsimd.dma_start(out=d_probe[0, 0:64, 0, 0:256],
                                        in_=t_SIG[:, :, :])
                    nc.gpsimd.dma_start(out=d_probe[0, 0:64, 2, 0:128],
                                        in_=t_TG[:, :])
                    nc.gpsimd.dma_start(out=d_probe[0, 0:64, 3, 0:64],
                                        in_=t_TC[:, :])
                    nc.gpsimd.dma_start(out=d_probe[0, 0:64, 4, 0:64],
                                        in_=t_HT[:, :])
            # post-loop: transpose the collected h tiles to [sample, du]
            for k in range(NT_OUT):
                pht = ltp.tile([64, 64], fp16, tag="pht", name="pht")
                nc.tensor.transpose(out=pht[:], in_=t_OUTT[:, k, :],
                                    identity=t_id128[0:64, 0:64])
                nc.vector.tensor_copy(out=t_OUT[:, k, :], in_=pht[:])

        nc.sync.dma_start(out=d_out[:, :, :], in_=t_OUT[:])

    nc.compile()
    return nc


def _get_nc(probe_layer=-1):
    key = ("nc", probe_layer)
    if key not in _cache:
        _cache[key] = _build(probe_layer)
    return _cache[key]


def run_on_cores(inputs, probe_layer=-1, trace=False):
    """Build (cached), run on 8 cores; returns (results, BassKernelResults)."""
    from concourse.bass_utils import run_bass_kernel_spmd

    nc = _get_nc(probe_layer)
    in_maps = _prep_host(inputs)
    last_exc = None
    for _ in range(3):
        try:
            res = run_bass_kernel_spmd(nc, in_maps,
                                       core_ids=list(range(N_CORES)),
                                       trace=trace)
            return res
        except Exception as e:  # transient NRT errors happen; retry
            last_exc = e
    raise last_exc


def assemble_output(res):
    out = np.zeros((B, NT_OUT, 64), np.float32)
    for core in range(N_CORES):
        s0 = core * S
        out[s0:s0 + S] = res.results[core]["out"]
    return out


def kernel(**inputs):
    res = run_on_cores(inputs)
    return assemble_output(res)



# revision 14
# speedup vs baseline: 1.0367x; 1.0148x over previous
"""Trainium2 Bass kernel for nn_Encoder_6 (conv+GN+InterpLnr x3 -> biLSTM).

Self-contained: host-side prep (sharding, interp gather tables, weight
repacking) + Bass/Tile device kernel + output gather.

Data-parallel over 8 NeuronCores: 64 samples per core.

Device dataflow per core (all samples resident on-chip after one load):
  - activations live in [channel(partition), sample, time] layout
  - conv1d = 10-11 accumulating matmuls per sample-pair (taps x cin-chunks),
    PSUM [128, 2x192]
  - GroupNorm stats fused into PSUM evacuation (ACT copy+accum -> sums,
    DVE square+accum -> sumsq), group reduce + expand via tiny matmuls
  - normalize+ReLU = single ACT op (per-partition scale/bias)
  - InterpLnr = banded-matrix matmul per sample (layer 2 also emits a
    time-reversed copy for the backward LSTM direction)
  - biLSTM in gate-major layout: state tiles are [64=(dir,unit), sample];
    per step: 4 tiny recurrence matmuls into the PSUM gate block, one
    sigmoid ACT over (i,f,2g) [tanh(g)=2*sig(2g)-1, the 2x folded into
    host weights], one sigmoid ACT for o, 3 DVE ops for the cell update,
    tanh ACT, 1 DVE for h.  No per-step transposes.
"""
import sys
from contextlib import ExitStack

sys.path.insert(0, "/opt/trn_rl_repo")

import numpy as np
import ml_dtypes

B = 512
N_CORES = 8
S = B // N_CORES          # samples per core
DIM_PIT = 257
C = 256                   # conv channels
T = 192                   # padded time
TH = 196                  # time with halo (2 each side)
GRP = 16                  # channels per group
DIM_NECK = 32
FREQ = 8
NT_OUT = 24               # output timesteps per direction
MIN_LEN_SEG = 19
MAX_NUM_SEG = 7
W64 = 64                  # 2*MAX_LEN_SEG
EPS = 1e-5
SG = 32                   # samples per stats group (2 groups per core)
NPAIR = 16                # sample pairs per stats group
LBLK = 2                  # LSTM timesteps per PSUM block (block = 1 PSUM bank)
NBLK = T // LBLK

_cache = {}


# ---------------------------------------------------------------- host prep

def _interp_tables(scales_u, len_seg_raw, n):
    """Gather idx/w1/w2 per sample for one interp layer (numpy, exact)."""
    scales = scales_u.astype(np.float32) + np.float32(0.5)
    j = np.arange(W64, dtype=np.float32)
    idx_scaled = j[None, :] / scales[:, None]
    idx_fl = np.floor(idx_scaled)
    lam = idx_scaled - idx_fl
    len_seg = (len_seg_raw + MIN_LEN_SEG).astype(np.float32)[:, None]
    idx_mask = idx_fl < (len_seg - 1.0)
    ls = (len_seg_raw + MIN_LEN_SEG).reshape(n, MAX_NUM_SEG)
    offset = np.cumsum(ls, axis=-1)
    offset = np.pad(offset[:, :-1], ((0, 0), (1, 0))).reshape(-1, 1)
    idx_org = idx_fl + offset.astype(np.float32)
    mask = (idx_mask & (idx_org < (T - 1))).reshape(n, MAX_NUM_SEG * W64)
    idx_b = np.clip(idx_org.reshape(n, -1).astype(np.int32), 0, T - 2)
    lam_b = lam.reshape(n, -1)
    idx = np.zeros((n, T), np.int32)
    w1 = np.zeros((n, T), np.float32)
    w2 = np.zeros((n, T), np.float32)
    for b in range(n):
        js = np.nonzero(mask[b])[0][:T]
        k = len(js)
        idx[b, :k] = idx_b[b, js]
        w1[b, :k] = 1.0 - lam_b[b, js]
        w2[b, :k] = lam_b[b, js]
    return idx, w1, w2


def _prep_host(inputs):
    """Build per-core input dicts. Returns list of 8 dicts."""
    x = np.asarray(inputs["x"], np.float32)
    scales = np.asarray(inputs["scales"], np.float32)
    lsr = np.asarray(inputs["len_seg_raw"], np.int32)

    # conv weights as lhsT tiles [l, chunk, tap, half, cin128, cout128]
    wconv = np.zeros((3, 2, 5, 2, 128, 128), np.float32)
    for l in range(3):
        w = np.asarray(inputs[f"conv{l}_w"], np.float32)  # [256, cin, 5]
        for cc in range(2):
            for k in range(5):
                for h in range(2):
                    wconv[l, cc, k, h] = w[h * 128:(h + 1) * 128,
                                           cc * 128:(cc + 1) * 128, k].T
    wconv = np.ascontiguousarray(wconv.astype(np.float16))
    # conv0 channel 256 as [5, 256] lhsT (k=tap)
    w0 = np.asarray(inputs["conv0_w"], np.float32)
    wc0e = np.ascontiguousarray(w0[:, 256, :].T.astype(np.float16))  # [5, 256]

    conv_bias = [np.asarray(inputs[f"conv{l}_b"], np.float32) for l in range(3)]
    assert all(np.abs(b).max() == 0.0 for b in conv_bias), \
        "nonzero conv bias not implemented in device kernel"

    gamma_t = np.stack([np.asarray(inputs[f"gn{l}_g"], np.float32).reshape(2, 128)
                        for l in range(3)])          # [3, 2, 128]
    beta_t = np.stack([np.asarray(inputs[f"gn{l}_b"], np.float32).reshape(2, 128)
                       for l in range(3)])
    gamma_t = np.ascontiguousarray(gamma_t.transpose(2, 0, 1).reshape(128, 6))
    beta_t = np.ascontiguousarray(beta_t.transpose(2, 0, 1).reshape(128, 6))

    gind = np.zeros((128, 8), np.float32)
    for c in range(128):
        gind[c, c // 16] = 1.0
    gexp = np.ascontiguousarray(gind.T)               # [8, 128]

    # interp tables, all samples
    idx_all, w1_all, w2_all = [], [], []
    for l in range(3):
        idx, w1, w2 = _interp_tables(scales[l], lsr[l], B)
        idx_all.append(idx)
        w1_all.append(w1)
        w2_all.append(w2)

    # LSTM weights, gate-major layout. gate order i,f,g,o; g-gate scaled
    # by 2 (tanh(g) = 2*sigmoid(2g) - 1 on device).
    #  wihG [128 cin, 4 gate, 2 cc, 64 (d,u)]  lhsT of xw matmuls
    #  whhG [64 (d,u'), 4 gate, 64 (d,u)]      lhsT of recurrence matmuls
    #  biasG [4 gate, 64 (d,u)]                lhsT of rank-1 bias matmuls
    H = DIM_NECK
    wihG = np.zeros((128, 4, 2, 64), np.float32)
    whhG = np.zeros((65, 4, 64), np.float32)   # row 64 = bias (ones in rhs)
    for d, nm in enumerate(["f", "b"]):
        wi = np.asarray(inputs[f"w_ih_{nm}"], np.float32)   # [128, 256]
        wh = np.asarray(inputs[f"w_hh_{nm}"], np.float32)   # [128, 32]
        bb = (np.asarray(inputs[f"b_ih_{nm}"], np.float32)
              + np.asarray(inputs[f"b_hh_{nm}"], np.float32))
        for g in range(4):
            sc = 2.0 if g == 2 else 1.0
            for cc in range(2):
                wihG[:, g, cc, d * H:(d + 1) * H] = \
                    sc * wi[g * H:(g + 1) * H, cc * 128:(cc + 1) * 128].T
            whhG[d * H:(d + 1) * H, g, d * H:(d + 1) * H] = \
                sc * wh[g * H:(g + 1) * H, :].T
            whhG[64, g, d * H:(d + 1) * H] = sc * bb[g * H:(g + 1) * H]
    wihG = np.ascontiguousarray(wihG.astype(np.float16))
    whhG = np.ascontiguousarray(whhG.astype(np.float16))

    in_maps = []
    for core in range(N_CORES):
        s0 = core * S
        xs = x[s0:s0 + S]                              # [S, 257, 192]
        xt = xs.transpose(1, 0, 2)                     # [257, S, 192]
        xab = np.zeros((128, 2, S, TH), np.float32)
        xab[:, 0, :, 2:194] = xt[:128]
        xab[:, 1, :, 2:194] = xt[128:256]
        xc = np.zeros((5, S, T), np.float32)
        x256 = xt[256]                                 # [S, 192]
        for k in range(5):
            sh = k - 2
            lo, hi = max(0, -sh), min(T, T - sh)
            xc[k, :, lo:hi] = x256[:, lo + sh:hi + sh]

        # banded interp matrices S[t_in, t_out] per (layer, sample), fp16
        wS = np.zeros((3, S, T, T), np.float16)
        bi = np.arange(S)[:, None]
        pj = np.arange(T)[None, :]
        for l in range(3):
            idx = idx_all[l][s0:s0 + S]
            Sm = np.zeros((S, T, T), np.float32)
            Sm[bi, idx, pj] = w1_all[l][s0:s0 + S]
            Sm[bi, idx + 1, pj] += w2_all[l][s0:s0 + S]
            wS[l] = Sm.astype(np.float16)

        in_maps.append({
            "xab": np.ascontiguousarray(xab.astype(np.float16)),
            "xc": np.ascontiguousarray(xc.astype(np.float16)),
            "wconv": wconv,
            "wc0e": wc0e,
            "gamma_t": gamma_t,
            "beta_t": beta_t,
            "gind": gind,
            "gexp": gexp,
            "wS": np.ascontiguousarray(wS),
            "id128": np.eye(128, dtype=np.float16),
            "wihG": wihG,
            "whhG": whhG,
        })
    return in_maps


# ------------------------------------------------------------- device build

def _build(probe_layer=-1):
    """Build the Bacc module. probe_layer >= 0 adds a probe output of XBUF
    after that layer's interp (for debugging)."""
    import concourse.bass as bass
    import concourse.tile as tile
    from concourse import bacc, mybir
    from concourse.masks import make_identity

    f32 = mybir.dt.float32
    f32r = mybir.dt.float32r
    bf16 = mybir.dt.bfloat16
    fp16 = mybir.dt.float16
    AF = mybir.ActivationFunctionType
    OP = mybir.AluOpType

    nc = bacc.Bacc("TRN2", target_bir_lowering=False, debug=False,
                   enable_asserts=False, num_devices=N_CORES)

    # DRAM tensors
    d_xab = nc.dram_tensor("xab", [128, 2, S, TH], fp16, kind="ExternalInput")
    d_xc = nc.dram_tensor("xc", [5, S, T], fp16, kind="ExternalInput")
    d_wconv = nc.dram_tensor("wconv", [3, 2, 5, 2, 128, 128], fp16,
                             kind="ExternalInput")
    d_wc0e = nc.dram_tensor("wc0e", [5, 256], fp16, kind="ExternalInput")
    d_gamma = nc.dram_tensor("gamma_t", [128, 6], f32, kind="ExternalInput")
    d_beta = nc.dram_tensor("beta_t", [128, 6], f32, kind="ExternalInput")
    d_gind = nc.dram_tensor("gind", [128, 8], f32, kind="ExternalInput")
    d_gexp = nc.dram_tensor("gexp", [8, 128], f32, kind="ExternalInput")
    d_wS = nc.dram_tensor("wS", [3, S, T, T], fp16, kind="ExternalInput")
    d_id128 = nc.dram_tensor("id128", [128, 128], fp16, kind="ExternalInput")
    d_wihG = nc.dram_tensor("wihG", [128, 4, 2, 64], fp16,
                            kind="ExternalInput")
    d_whhG = nc.dram_tensor("whhG", [65, 4, 64], fp16, kind="ExternalInput")
    d_out = nc.dram_tensor("out", [S, NT_OUT, 64], f32, kind="ExternalOutput")
    d_probe = None
    if probe_layer >= 0:
        d_probe = nc.dram_tensor("probe", [2, 128, S, TH], f32r,
                                 kind="ExternalOutput")

    es = ExitStack()
    with tile.TileContext(nc) as tc, es:
        consts = es.enter_context(tc.tile_pool(name="consts", bufs=1))
        xbufs = es.enter_context(tc.tile_pool(name="xbufs", bufs=1))

        # ---- constants
        t_xc = consts.tile([5, S, T], fp16)
        nc.sync.dma_start(out=t_xc[:], in_=d_xc[:, :, :])
        t_wc0e = consts.tile([5, 256], fp16)
        nc.sync.dma_start(out=t_wc0e[:], in_=d_wc0e[:, :])
        t_gamma = consts.tile([128, 6], f32)
        nc.sync.dma_start(out=t_gamma[:], in_=d_gamma[:, :])
        t_beta = consts.tile([128, 6], f32)
        nc.sync.dma_start(out=t_beta[:], in_=d_beta[:, :])
        t_gind = consts.tile([128, 8], f32)
        nc.sync.dma_start(out=t_gind[:], in_=d_gind[:, :])
        t_gexp = consts.tile([8, 128], f32)
        nc.sync.dma_start(out=t_gexp[:], in_=d_gexp[:, :])
        t_eps = consts.tile([8, 1], f32)
        nc.vector.memset(t_eps[:], EPS)
        t_id128 = consts.tile([128, 128], fp16)
        nc.sync.dma_start(out=t_id128[:], in_=d_id128[:, :])
        # LSTM consts
        t_wihG = consts.tile([128, 4, 2, 64], fp16)
        nc.sync.dma_start(out=t_wihG[:], in_=d_wihG[:, :, :, :])
        t_whhG = consts.tile([65, 4, 64], fp16)
        nc.sync.dma_start(out=t_whhG[:], in_=d_whhG[:, :, :])
        t_ones64h = consts.tile([64, 64], fp16)
        nc.vector.memset(t_ones64h[:], 1.0)

        # ---- input activations (xbuf reused as interp output every layer)
        # chunked DMA so the first conv matmuls start after ~1/4 of the load
        t_x = xbufs.tile([128, 2, S, TH], fp16)
        for sc in range(0, S, 16):
            nc.sync.dma_start(out=t_x[:, :, sc:sc + 16, :],
                              in_=d_xab[:, :, sc:sc + 16, :])
        # layer-2 interp output in [c, cc, t, s] layout: LSTM xw matmuls read
        # contiguous (t, s) runs instead of stride-196 sample-major slices
        t_xT = xbufs.tile([128, 2, T, S], fp16)

        def mm(out, lhsT, rhs, start, stop, dt=None, **kw):
            if dt is not None:
                lhsT = lhsT.bitcast(dt)
                rhs = rhs.bitcast(dt)
            nc.tensor.matmul(out=out, lhsT=lhsT, rhs=rhs, start=start,
                             stop=stop, **kw)

        # ================= conv + GN + interp layers =================
        with ExitStack() as ces:
            wpool = ces.enter_context(tc.tile_pool(name="wpool", bufs=1))
            hraw_p = ces.enter_context(tc.tile_pool(name="hraw", bufs=2))
            stats_p = ces.enter_context(tc.tile_pool(name="stats", bufs=2))
            small_p = ces.enter_context(tc.tile_pool(name="small", bufs=2))
            y_p = ces.enter_context(tc.tile_pool(name="ybuf", bufs=3))
            scr_p = ces.enter_context(tc.tile_pool(name="scr", bufs=3))
            sm_p = ces.enter_context(tc.tile_pool(name="smat", bufs=2))
            yt_p = ces.enter_context(tc.tile_pool(name="ytp", bufs=3))
            cpsum = ces.enter_context(
                tc.tile_pool(name="cpsum", bufs=2, space="PSUM"))
            stps = ces.enter_context(
                tc.tile_pool(name="stps", bufs=1, space="PSUM"))
            tpsum = ces.enter_context(
                tc.tile_pool(name="tpsum", bufs=2, space="PSUM"))
            sops = ces.enter_context(
                tc.tile_pool(name="sops", bufs=3, space="PSUM"))

            for l in range(3):
                t_wc = wpool.tile([128, 20, 128], fp16, tag="wconv")
                nc.sync.dma_start(
                    out=t_wc[:],
                    in_=bass.AP(tensor=d_wconv, offset=l * 20 * 128 * 128,
                                ap=[[128, 128], [128 * 128, 20], [1, 128]]))

                for grp in range(2):
                    sums = [stats_p.tile([128, SG], f32, tag=f"sums{h}", name=f"sums{h}")
                            for h in range(2)]
                    qs = [stats_p.tile([128, SG], f32, tag=f"qs{h}", name=f"qs{h}")
                          for h in range(2)]
                    hraw = [hraw_p.tile([128, SG, T], fp16, tag=f"hraw{h}", name=f"hraw{h}")
                            for h in range(2)]

                    # ---- phase 1: conv + fused stats
                    for pp in range(NPAIR):
                        pr = grp * NPAIR + pp
                        for h in range(2):
                            ps = cpsum.tile([128, 2, T], f32, tag="cps")
                            ops = []
                            for cc in range(2):
                                for k in range(5):
                                    ops.append((
                                        t_wc[:, (cc * 5 + k) * 2 + h, :],
                                        t_x[:, cc, 2 * pr:2 * pr + 2,
                                            k:k + T], None))
                            if l == 0:
                                ops.append((
                                    t_wc0e[:, h * 128:(h + 1) * 128],
                                    t_xc[:, 2 * pr:2 * pr + 2, :], None))
                            for j, (lh, rh, dt) in enumerate(ops):
                                mm(ps[:], lh, rh, j == 0, j == len(ops) - 1,
                                   dt=dt)
                            for i in range(2):
                                sl = pp * 2 + i
                                nc.scalar.activation(
                                    out=hraw[h][:, sl, :], in_=ps[:, i, :],
                                    func=AF.Identity,
                                    accum_out=sums[h][:, sl:sl + 1])
                                scr = scr_p.tile([128, T], fp16, tag="sq")
                                nc.vector.scalar_tensor_tensor(
                                    out=scr[:], in0=hraw[h][:, sl, :],
                                    scalar=1.0,
                                    in1=hraw[h][:, sl, :], op0=OP.mult,
                                    op1=OP.mult,
                                    accum_out=qs[h][:, sl:sl + 1])

                    # ---- phase 2: group stats -> A, B per half
                    AB = []
                    for h in range(2):
                        g1 = stps.tile([8, SG], f32, tag="gg")
                        mm(g1[:], t_gind[:], sums[h][:], True, True)
                        g2 = stps.tile([8, SG], f32, tag="gg")
                        mm(g2[:], t_gind[:], qs[h][:], True, True)
                        mean = small_p.tile([8, SG], f32, tag="mean")
                        nc.vector.tensor_scalar_mul(mean[:], g1[:],
                                                    1.0 / (GRP * T))
                        msq = small_p.tile([8, SG], f32, tag="msq")
                        nc.vector.tensor_tensor(out=msq[:], in0=mean[:],
                                                in1=mean[:], op=OP.mult)
                        var = small_p.tile([8, SG], f32, tag="var")
                        nc.vector.scalar_tensor_tensor(
                            out=var[:], in0=g2[:], scalar=1.0 / (GRP * T),
                            in1=msq[:], op0=OP.mult, op1=OP.subtract)
                        sd = small_p.tile([8, SG], f32, tag="sd")
                        nc.scalar.activation(out=sd[:], in_=var[:],
                                             func=AF.Sqrt,
                                             bias=t_eps[:, :1], scale=1.0)
                        rstd = small_p.tile([8, SG], f32, tag="rstd")
                        nc.vector.reciprocal(rstd[:], sd[:])
                        rp = stps.tile([128, SG], f32, tag="gg")
                        mm(rp[:], t_gexp[:], rstd[:], True, True)
                        mp = stps.tile([128, SG], f32, tag="gg")
                        mm(mp[:], t_gexp[:], mean[:], True, True)
                        At = small_p.tile([128, SG], f32, tag="A")
                        nc.vector.tensor_scalar_mul(
                            At[:], rp[:],
                            t_gamma[:, l * 2 + h:l * 2 + h + 1])
                        tmp = small_p.tile([128, SG], f32, tag="tmp")
                        nc.vector.tensor_tensor(out=tmp[:], in0=mp[:],
                                                in1=At[:], op=OP.mult)
                        Bt = small_p.tile([128, SG], f32, tag="B")
                        nc.vector.tensor_scalar(
                            out=Bt[:], in0=tmp[:], scalar1=-1.0,
                            scalar2=t_beta[:, l * 2 + h:l * 2 + h + 1],
                            op0=OP.mult, op1=OP.add)
                        AB.append((At, Bt))

                    # ---- phase 3: normalize+relu, transpose, interp matmul
                    for pp in range(NPAIR):
                        pr = grp * NPAIR + pp
                        s128 = sm_p.tile([128, 2, T], fp16, tag="s128")
                        nc.gpsimd.dma_start(
                            out=s128[:],
                            in_=d_wS[l, 2 * pr:2 * pr + 2, 0:128, :].rearrange(
                                "s t w -> t s w"))
                        s64 = sm_p.tile([64, 2, T], fp16, tag="s64")
                        nc.gpsimd.dma_start(
                            out=s64[:],
                            in_=d_wS[l, 2 * pr:2 * pr + 2, 128:192, :].rearrange(
                                "s t w -> t s w"))
                        for i in range(2):
                            sl = pp * 2 + i
                            sg_ = 2 * pr + i
                            yth = [yt_p.tile([128, 2, 128], fp16,
                                             tag=f"yth{h}", name=f"yth{h}")
                                   for h in range(2)]
                            for h in range(2):
                                At, Bt = AB[h]
                                ytmp = y_p.tile([128, T], fp16, tag="ytmp")
                                nc.vector.tensor_scalar(
                                    out=ytmp[:], in0=hraw[h][:, sl, :],
                                    scalar1=At[:, sl:sl + 1],
                                    scalar2=Bt[:, sl:sl + 1],
                                    op0=OP.mult, op1=OP.add)
                                yb = y_p.tile([128, T], fp16, tag="yb")
                                nc.vector.tensor_scalar_max(
                                    yb[:], ytmp[:], 0.0)
                                ptp = tpsum.tile([128, 2, 128], fp16,
                                                 tag="tp", name="ptp")
                                nc.tensor.transpose(
                                    out=ptp[:, 0, :], in_=yb[:, 0:128],
                                    identity=t_id128[:])
                                nc.tensor.transpose(
                                    out=ptp[0:64, 1, :], in_=yb[:, 128:192],
                                    identity=t_id128[:])
                                nc.vector.tensor_copy(
                                    out=yth[h][:], in_=ptp[:, :, :])
                            sout = sops.tile([128, 2, T], f32, tag="so",
                                             name="sout")
                            for ch in range(2):
                                mm(sout[:, ch, :], yth[ch][:, 0, :],
                                   s128[:, i, :], True, False)
                                mm(sout[:, ch, :], yth[ch][0:64, 1, :],
                                   s64[:, i, :], False, True)
                            if l < 2:
                                nc.vector.tensor_copy(
                                    out=t_x[:, :, sg_, 2:194],
                                    in_=sout[:, :, :])
                            else:
                                nc.vector.tensor_copy(
                                    out=t_xT[:, :, :, sg_],
                                    in_=sout[:, :, :])

                if probe_layer == l:
                    for h in range(2):
                        nc.gpsimd.dma_start(out=d_probe[h, :, :, :],
                                            in_=t_x[:, h, :, :])

        # ======================= biLSTM (gate-major) =======================
        # state tiles [64 part=(dir,unit), 64 cols=sample]; gates in PSUM
        # blocks [64, LBLK, 4(gate i,f,g,o), 64], one PSUM bank per block.
        lsb = es.enter_context(tc.tile_pool(name="lstm_sbuf", bufs=1))
        t_SIG = lsb.tile([64, 4, 64], fp16, name="sig")  # sig(i,f,2g,o)
        t_Q = lsb.tile([64, 64], fp16, name="qq")        # (sig2g-.5)*si
        t_R = lsb.tile([64, 64], fp16, name="rr")        # sf*c
        t_C = lsb.tile([64, 64], fp16, name="cc")        # cell state
        t_TC = lsb.tile([64, 64], fp16, name="tc")       # tanh(c)
        t_HT = lsb.tile([65, 64], fp16, name="ht")       # h; row 64 = ones
        t_OUT = lsb.tile([64, NT_OUT, 64], f32, name="outt")
        nc.vector.memset(t_C[:], 0.0)
        nc.vector.memset(t_HT[0:64, :], 0.0)
        nc.vector.memset(t_HT[64:65, :], 1.0)

        with tc.tile_pool(name="lpsum", bufs=2, space="PSUM") as lpsum, \
             tc.tile_pool(name="ltp", bufs=2, space="PSUM") as ltp:

            def xw_half(blk, half, G=None):
                """xw preacts for gates [2*half, 2*half+2) of one block.
                start=True clears the full bank row for the partitions the
                matmul writes -> one start per direction, on its first
                matmul (half 0)."""
                t0 = blk * LBLK
                if G is None:
                    G = lpsum.tile([64, LBLK, 4, 64], f32, tag="xw",
                                   name="xw")
                for g4 in range(2 * half, 2 * half + 2):
                    for cc in range(2):
                        # dir f reads time t0..t0+LBLK-1 (contiguous (t, s))
                        rf = t_xT[:, cc, t0:t0 + LBLK, :]
                        mm(G[0:32, :, g4, :], t_wihG[:, g4, cc, 0:32], rf,
                           g4 == 2 * half == 0 and cc == 0, False)
                        # dir b reads time 191-t0 downward (negative t stride)
                        base = t_xT[:, cc, 0:LBLK, :]
                        rb = bass.AP(tensor=base.tensor,
                                     offset=base.offset + (191 - t0) * S,
                                     ap=[base.ap[0], [-S, LBLK], [1, S]])
                        mm(G[32:64, :, g4, :], t_wihG[:, g4, cc, 32:64], rb,
                           g4 == 2 * half == 0 and cc == 0, False)
                return G

            xwp = [xw_half(0, 1, xw_half(0, 0)), None]
            t_OUTT = lsb.tile([64, NT_OUT, 64], fp16, name="outth")
            if probe_layer == 4:
                t_gdbg = lsb.tile([64, LBLK, 4, 64], f32, name="gdbg")
                nc.vector.tensor_copy(out=t_gdbg[:], in_=xwp[0][:, :, :, :])
                nc.gpsimd.dma_start(out=d_probe[0, 0:64, 0:LBLK * 4, 0:64],
                                    in_=t_gdbg[:, :, :, :])

            for g in range(T):
                blk, j = g // LBLK, g % LBLK
                G = xwp[blk % 2]
                # i, f, g rec matmuls first; sigmoid(i,f,2g) can then start
                # while the o-gate matmul + sigmoid run off the critical path
                for g4 in range(3):
                    mm(G[:, j, g4, :], t_whhG[:, g4, :], t_HT[:],
                       False, True, skip_group_check=True)
                nc.scalar.activation(
                    out=t_SIG[:, 0:3, :], in_=G[:, j, 0:3, :],
                    func=AF.Sigmoid)
                mm(G[:, j, 3, :], t_whhG[:, 3, :], t_HT[:],
                   False, True, skip_group_check=True)
                nc.scalar.activation(
                    out=t_SIG[:, 3, :], in_=G[:, j, 3, :],
                    func=AF.Sigmoid)
                # q = (sig(2g) - 0.5) * sig(i)  [= sig(i)*tanh(g)/2]
                nc.vector.scalar_tensor_tensor(
                    out=t_Q[:], in0=t_SIG[:, 2, :], scalar=0.5,
                    in1=t_SIG[:, 0, :], op0=OP.subtract, op1=OP.mult)
                # r = sig(f) * c  (gpsimd, parallel with q)
                nc.gpsimd.tensor_tensor(out=t_R[:], in0=t_SIG[:, 1, :],
                                        in1=t_C[:], op=OP.mult)
                # c = 2*q + r
                nc.vector.scalar_tensor_tensor(
                    out=t_C[:], in0=t_Q[:], scalar=2.0,
                    in1=t_R[:], op0=OP.mult, op1=OP.add)
                nc.scalar.activation(out=t_TC[:], in_=t_C[:],
                                     func=AF.Tanh)
                nc.vector.tensor_tensor(out=t_HT[0:64, :],
                                        in0=t_SIG[:, 3, :],
                                        in1=t_TC[:], op=OP.mult)
                if g % FREQ == FREQ - 1:
                    nc.vector.tensor_copy(
                        out=t_OUTT[0:32, g // FREQ, :], in_=t_HT[0:32, :])
                    nc.vector.tensor_copy(
                        out=t_OUTT[32:64, (T - 1 - g) // FREQ, :],
                        in_=t_HT[32:64, :])
                if j == 0 and blk + 1 < NBLK:
                    xwp[(blk + 1) % 2] = xw_half(blk + 1, 0)
                if j == 1 and blk + 1 < NBLK:
                    xw_half(blk + 1, 1, xwp[(blk + 1) % 2])
                if probe_layer == 5 and g == 0:
                    nc.gpsimd.dma_start(out=d_probe[0, 0:64, 0, 0:256],
                                        in_=t_SIG[:, :, :])
                    nc.gpsimd.dma_start(out=d_probe[0, 0:64, 3, 0:64],
                                        in_=t_TC[:, :])
                    nc.gpsimd.dma_start(out=d_probe[0, 0:64, 4, 0:64],
                                        in_=t_HT[:, :])
            # post-loop: transpose the collected h tiles to [sample, du]
            for k in range(NT_OUT):
                pht = ltp.tile([64, 64], fp16, tag="pht", name="pht")
                nc.tensor.transpose(out=pht[:], in_=t_OUTT[:, k, :],
                                    identity=t_id128[0:64, 0:64])
                nc.vector.tensor_copy(out=t_OUT[:, k, :], in_=pht[:])

        nc.sync.dma_start(out=d_out[:, :, :], in_=t_OUT[:])

    nc.compile()
    return nc


def _get_nc(probe_layer=-1):
    key = ("nc", probe_layer)
    if key not in _cache:
        _cache[key] = _build(probe_layer)
    return _cache[key]


def run_on_cores(inputs, probe_layer=-1, trace=False):
    """Build (cached), run on 8 cores; returns (results, BassKernelResults)."""
    from concourse.bass_utils import run_bass_kernel_spmd

    nc = _get_nc(probe_layer)
    in_maps = _prep_host(inputs)
    last_exc = None
    for _ in range(3):
        try:
            res = run_bass_kernel_spmd(nc, in_maps,
                                       core_ids=list(range(N_CORES)),
                                       trace=trace)
            return res
        except Exception as e:  # transient NRT errors happen; retry
            last_exc = e
    raise last_exc


def assemble_output(res):
    out = np.zeros((B, NT_OUT, 64), np.float32)
    for core in range(N_CORES):
        s0 = core * S
        out[s0:s0 + S] = res.results[core]["out"]
    return out


def kernel(**inputs):
    res = run_on_cores(inputs)
    return assemble_output(res)



# revision 17
# speedup vs baseline: 1.0526x; 1.0154x over previous
"""Trainium2 Bass kernel for nn_Encoder_6 (conv+GN+InterpLnr x3 -> biLSTM).

Self-contained: host-side prep (sharding, interp gather tables, weight
repacking) + Bass/Tile device kernel + output gather.

Data-parallel over 8 NeuronCores: 64 samples per core.

Device dataflow per core (all samples resident on-chip after one load):
  - activations live in [channel(partition), sample, time] layout
  - conv1d = 10-11 accumulating matmuls per sample-pair (taps x cin-chunks),
    PSUM [128, 2x192]
  - GroupNorm stats fused into PSUM evacuation (ACT copy+accum -> sums,
    DVE square+accum -> sumsq), group reduce + expand via tiny matmuls
  - normalize+ReLU = single ACT op (per-partition scale/bias)
  - InterpLnr = banded-matrix matmul per sample (layer 2 also emits a
    time-reversed copy for the backward LSTM direction)
  - biLSTM in gate-major layout: state tiles are [64=(dir,unit), sample];
    per step: 4 tiny recurrence matmuls into the PSUM gate block, one
    sigmoid ACT over (i,f,2g) [tanh(g)=2*sig(2g)-1, the 2x folded into
    host weights], one sigmoid ACT for o, 3 DVE ops for the cell update,
    tanh ACT, 1 DVE for h.  No per-step transposes.
"""
import sys
from contextlib import ExitStack

sys.path.insert(0, "/opt/trn_rl_repo")

import numpy as np
import ml_dtypes

B = 512
N_CORES = 8
S = B // N_CORES          # samples per core
DIM_PIT = 257
C = 256                   # conv channels
T = 192                   # padded time
TH = 196                  # time with halo (2 each side)
GRP = 16                  # channels per group
DIM_NECK = 32
FREQ = 8
NT_OUT = 24               # output timesteps per direction
MIN_LEN_SEG = 19
MAX_NUM_SEG = 7
W64 = 64                  # 2*MAX_LEN_SEG
EPS = 1e-5
SG = 32                   # samples per stats group (2 groups per core)
NPAIR = 16                # sample pairs per stats group
LBLK = 2                  # LSTM timesteps per PSUM block (block = 1 PSUM bank)
NBLK = T // LBLK

_cache = {}


# ---------------------------------------------------------------- host prep

def _interp_tables(scales_u, len_seg_raw, n):
    """Gather idx/w1/w2 per sample for one interp layer (numpy, exact)."""
    scales = scales_u.astype(np.float32) + np.float32(0.5)
    j = np.arange(W64, dtype=np.float32)
    idx_scaled = j[None, :] / scales[:, None]
    idx_fl = np.floor(idx_scaled)
    lam = idx_scaled - idx_fl
    len_seg = (len_seg_raw + MIN_LEN_SEG).astype(np.float32)[:, None]
    idx_mask = idx_fl < (len_seg - 1.0)
    ls = (len_seg_raw + MIN_LEN_SEG).reshape(n, MAX_NUM_SEG)
    offset = np.cumsum(ls, axis=-1)
    offset = np.pad(offset[:, :-1], ((0, 0), (1, 0))).reshape(-1, 1)
    idx_org = idx_fl + offset.astype(np.float32)
    mask = (idx_mask & (idx_org < (T - 1))).reshape(n, MAX_NUM_SEG * W64)
    idx_b = np.clip(idx_org.reshape(n, -1).astype(np.int32), 0, T - 2)
    lam_b = lam.reshape(n, -1)
    idx = np.zeros((n, T), np.int32)
    w1 = np.zeros((n, T), np.float32)
    w2 = np.zeros((n, T), np.float32)
    for b in range(n):
        js = np.nonzero(mask[b])[0][:T]
        k = len(js)
        idx[b, :k] = idx_b[b, js]
        w1[b, :k] = 1.0 - lam_b[b, js]
        w2[b, :k] = lam_b[b, js]
    return idx, w1, w2


def _prep_host(inputs):
    """Build per-core input dicts. Returns list of 8 dicts."""
    x = np.asarray(inputs["x"], np.float32)
    scales = np.asarray(inputs["scales"], np.float32)
    lsr = np.asarray(inputs["len_seg_raw"], np.int32)

    # conv weights as lhsT tiles [l, chunk, tap, half, cin128, cout128]
    wconv = np.zeros((3, 2, 5, 2, 128, 128), np.float32)
    for l in range(3):
        w = np.asarray(inputs[f"conv{l}_w"], np.float32)  # [256, cin, 5]
        for cc in range(2):
            for k in range(5):
                for h in range(2):
                    wconv[l, cc, k, h] = w[h * 128:(h + 1) * 128,
                                           cc * 128:(cc + 1) * 128, k].T
    wconv = np.ascontiguousarray(wconv.astype(np.float16))
    # conv0 channel 256 as [5, 256] lhsT (k=tap)
    w0 = np.asarray(inputs["conv0_w"], np.float32)
    wc0e = np.ascontiguousarray(w0[:, 256, :].T.astype(np.float16))  # [5, 256]

    conv_bias = [np.asarray(inputs[f"conv{l}_b"], np.float32) for l in range(3)]
    assert all(np.abs(b).max() == 0.0 for b in conv_bias), \
        "nonzero conv bias not implemented in device kernel"

    gamma_t = np.stack([np.asarray(inputs[f"gn{l}_g"], np.float32).reshape(2, 128)
                        for l in range(3)])          # [3, 2, 128]
    beta_t = np.stack([np.asarray(inputs[f"gn{l}_b"], np.float32).reshape(2, 128)
                       for l in range(3)])
    gamma_t = np.ascontiguousarray(gamma_t.transpose(2, 0, 1).reshape(128, 6))
    beta_t = np.ascontiguousarray(beta_t.transpose(2, 0, 1).reshape(128, 6))

    gind = np.zeros((128, 8), np.float32)
    for c in range(128):
        gind[c, c // 16] = 1.0
    gexp = np.ascontiguousarray(gind.T)               # [8, 128]

    # interp tables, all samples
    idx_all, w1_all, w2_all = [], [], []
    for l in range(3):
        idx, w1, w2 = _interp_tables(scales[l], lsr[l], B)
        idx_all.append(idx)
        w1_all.append(w1)
        w2_all.append(w2)

    # LSTM weights, gate-major layout. gate order i,f,g,o; g-gate scaled
    # by 2 (tanh(g) = 2*sigmoid(2g) - 1 on device).
    #  wihG [128 cin, 4 gate, 2 cc, 64 (d,u)]  lhsT of xw matmuls
    #  whhG [64 (d,u'), 4 gate, 64 (d,u)]      lhsT of recurrence matmuls
    #  biasG [4 gate, 64 (d,u)]                lhsT of rank-1 bias matmuls
    H = DIM_NECK
    wihG = np.zeros((128, 4, 2, 64), np.float32)
    whhG = np.zeros((65, 4, 64), np.float32)   # row 64 = bias (ones in rhs)
    for d, nm in enumerate(["f", "b"]):
        wi = np.asarray(inputs[f"w_ih_{nm}"], np.float32)   # [128, 256]
        wh = np.asarray(inputs[f"w_hh_{nm}"], np.float32)   # [128, 32]
        bb = (np.asarray(inputs[f"b_ih_{nm}"], np.float32)
              + np.asarray(inputs[f"b_hh_{nm}"], np.float32))
        for g in range(4):
            sc = 2.0 if g == 2 else 1.0
            for cc in range(2):
                wihG[:, g, cc, d * H:(d + 1) * H] = \
                    sc * wi[g * H:(g + 1) * H, cc * 128:(cc + 1) * 128].T
            whhG[d * H:(d + 1) * H, g, d * H:(d + 1) * H] = \
                sc * wh[g * H:(g + 1) * H, :].T
            whhG[64, g, d * H:(d + 1) * H] = sc * bb[g * H:(g + 1) * H]
    wihG = np.ascontiguousarray(wihG.astype(np.float16))
    whhG = np.ascontiguousarray(whhG.astype(np.float16))

    in_maps = []
    for core in range(N_CORES):
        s0 = core * S
        xs = x[s0:s0 + S]                              # [S, 257, 192]
        xt = xs.transpose(1, 0, 2)                     # [257, S, 192]
        xab = np.zeros((128, 2, S, TH), np.float32)
        xab[:, 0, :, 2:194] = xt[:128]
        xab[:, 1, :, 2:194] = xt[128:256]
        xc = np.zeros((5, S, T), np.float32)
        x256 = xt[256]                                 # [S, 192]
        for k in range(5):
            sh = k - 2
            lo, hi = max(0, -sh), min(T, T - sh)
            xc[k, :, lo:hi] = x256[:, lo + sh:hi + sh]

        # banded interp matrices S[t_in, t_out] per (layer, sample), fp16
        wS = np.zeros((3, S, T, T), np.float16)
        bi = np.arange(S)[:, None]
        pj = np.arange(T)[None, :]
        for l in range(3):
            idx = idx_all[l][s0:s0 + S]
            Sm = np.zeros((S, T, T), np.float32)
            Sm[bi, idx, pj] = w1_all[l][s0:s0 + S]
            Sm[bi, idx + 1, pj] += w2_all[l][s0:s0 + S]
            wS[l] = Sm.astype(np.float16)

        in_maps.append({
            "xab": np.ascontiguousarray(xab.astype(np.float16)),
            "xc": np.ascontiguousarray(xc.astype(np.float16)),
            "wconv": wconv,
            "wc0e": wc0e,
            "gamma_t": gamma_t,
            "beta_t": beta_t,
            "gind": gind,
            "gexp": gexp,
            "wS": np.ascontiguousarray(wS),
            "id128": np.eye(128, dtype=np.float16),
            "wihG": wihG,
            "whhG": whhG,
        })
    return in_maps


# ------------------------------------------------------------- device build

def _build(probe_layer=-1):
    """Build the Bacc module. probe_layer >= 0 adds a probe output of XBUF
    after that layer's interp (for debugging)."""
    import concourse.bass as bass
    import concourse.tile as tile
    from concourse import bacc, mybir
    from concourse.masks import make_identity

    f32 = mybir.dt.float32
    f32r = mybir.dt.float32r
    bf16 = mybir.dt.bfloat16
    fp16 = mybir.dt.float16
    AF = mybir.ActivationFunctionType
    OP = mybir.AluOpType

    nc = bacc.Bacc("TRN2", target_bir_lowering=False, debug=False,
                   enable_asserts=False, num_devices=N_CORES)

    # DRAM tensors
    d_xab = nc.dram_tensor("xab", [128, 2, S, TH], fp16, kind="ExternalInput")
    d_xc = nc.dram_tensor("xc", [5, S, T], fp16, kind="ExternalInput")
    d_wconv = nc.dram_tensor("wconv", [3, 2, 5, 2, 128, 128], fp16,
                             kind="ExternalInput")
    d_wc0e = nc.dram_tensor("wc0e", [5, 256], fp16, kind="ExternalInput")
    d_gamma = nc.dram_tensor("gamma_t", [128, 6], f32, kind="ExternalInput")
    d_beta = nc.dram_tensor("beta_t", [128, 6], f32, kind="ExternalInput")
    d_gind = nc.dram_tensor("gind", [128, 8], f32, kind="ExternalInput")
    d_gexp = nc.dram_tensor("gexp", [8, 128], f32, kind="ExternalInput")
    d_wS = nc.dram_tensor("wS", [3, S, T, T], fp16, kind="ExternalInput")
    d_id128 = nc.dram_tensor("id128", [128, 128], fp16, kind="ExternalInput")
    d_wihG = nc.dram_tensor("wihG", [128, 4, 2, 64], fp16,
                            kind="ExternalInput")
    d_whhG = nc.dram_tensor("whhG", [65, 4, 64], fp16, kind="ExternalInput")
    d_out = nc.dram_tensor("out", [S, NT_OUT, 64], f32, kind="ExternalOutput")
    d_probe = None
    if probe_layer >= 0:
        d_probe = nc.dram_tensor("probe", [2, 128, S, TH], f32r,
                                 kind="ExternalOutput")

    es = ExitStack()
    with tile.TileContext(nc) as tc, es:
        consts = es.enter_context(tc.tile_pool(name="consts", bufs=1))
        xbufs = es.enter_context(tc.tile_pool(name="xbufs", bufs=1))

        # ---- constants
        t_xc = consts.tile([5, S, T], fp16)
        nc.sync.dma_start(out=t_xc[:], in_=d_xc[:, :, :])
        t_wc0e = consts.tile([5, 256], fp16)
        nc.sync.dma_start(out=t_wc0e[:], in_=d_wc0e[:, :])
        t_gamma = consts.tile([128, 6], f32)
        nc.sync.dma_start(out=t_gamma[:], in_=d_gamma[:, :])
        t_beta = consts.tile([128, 6], f32)
        nc.sync.dma_start(out=t_beta[:], in_=d_beta[:, :])
        t_gind = consts.tile([128, 8], f32)
        nc.sync.dma_start(out=t_gind[:], in_=d_gind[:, :])
        t_gexp = consts.tile([8, 128], f32)
        nc.sync.dma_start(out=t_gexp[:], in_=d_gexp[:, :])
        t_eps = consts.tile([8, 1], f32)
        nc.vector.memset(t_eps[:], EPS)
        t_id128 = consts.tile([128, 128], fp16)
        nc.sync.dma_start(out=t_id128[:], in_=d_id128[:, :])
        # LSTM consts
        t_wihG = consts.tile([128, 4, 2, 64], fp16)
        nc.sync.dma_start(out=t_wihG[:], in_=d_wihG[:, :, :, :])
        t_whhG = consts.tile([65, 4, 64], fp16)
        nc.sync.dma_start(out=t_whhG[:], in_=d_whhG[:, :, :])
        t_ones64h = consts.tile([64, 64], fp16)
        nc.vector.memset(t_ones64h[:], 1.0)

        t_x = xbufs.tile([128, 2, S, TH], fp16)
        # layer-2 interp output in [c, cc, t, s] layout: LSTM xw matmuls read
        # contiguous (t, s) runs instead of stride-196 sample-major slices
        t_xT = xbufs.tile([128, 2, T, S], fp16)

        def mm(out, lhsT, rhs, start, stop, dt=None, **kw):
            if dt is not None:
                lhsT = lhsT.bitcast(dt)
                rhs = rhs.bitcast(dt)
            nc.tensor.matmul(out=out, lhsT=lhsT, rhs=rhs, start=start,
                             stop=stop, **kw)

        # ================= conv + GN + interp layers =================
        with ExitStack() as ces:
            wpool = ces.enter_context(tc.tile_pool(name="wpool", bufs=1))

            def load_wconv(l):
                t_wc = wpool.tile([128, 20, 128], fp16, tag="wconv")
                nc.sync.dma_start(
                    out=t_wc[:],
                    in_=bass.AP(tensor=d_wconv, offset=l * 20 * 128 * 128,
                                ap=[[128, 128], [128 * 128, 20], [1, 128]]))
                return t_wc

            # layer-0 weights first so the first conv matmuls only wait for
            # the small weight DMA + the first input chunk
            t_wc0 = load_wconv(0)
            # ---- input activations (xbuf reused as interp output per layer);
            # chunked DMA so early pairs start before the full load finishes
            for sc in range(0, S, 16):
                nc.sync.dma_start(out=t_x[:, :, sc:sc + 16, :],
                                  in_=d_xab[:, :, sc:sc + 16, :])
            hraw_p = ces.enter_context(tc.tile_pool(name="hraw", bufs=2))
            stats_p = ces.enter_context(tc.tile_pool(name="stats", bufs=2))
            small_p = ces.enter_context(tc.tile_pool(name="small", bufs=2))
            y_p = ces.enter_context(tc.tile_pool(name="ybuf", bufs=3))
            scr_p = ces.enter_context(tc.tile_pool(name="scr", bufs=3))
            sm_p = ces.enter_context(tc.tile_pool(name="smat", bufs=2))
            yt_p = ces.enter_context(tc.tile_pool(name="ytp", bufs=3))
            cpsum = ces.enter_context(
                tc.tile_pool(name="cpsum", bufs=2, space="PSUM"))
            stps = ces.enter_context(
                tc.tile_pool(name="stps", bufs=1, space="PSUM"))
            tpsum = ces.enter_context(
                tc.tile_pool(name="tpsum", bufs=2, space="PSUM"))
            sops = ces.enter_context(
                tc.tile_pool(name="sops", bufs=3, space="PSUM"))

            for l in range(3):
                t_wc = t_wc0 if l == 0 else load_wconv(l)

                for grp in range(2):
                    sums = [stats_p.tile([128, SG], f32, tag=f"sums{h}", name=f"sums{h}")
                            for h in range(2)]
                    qs = [stats_p.tile([128, SG], f32, tag=f"qs{h}", name=f"qs{h}")
                          for h in range(2)]
                    hraw = [hraw_p.tile([128, SG, T], fp16, tag=f"hraw{h}", name=f"hraw{h}")
                            for h in range(2)]

                    # ---- phase 1: conv + fused stats
                    for pp in range(NPAIR):
                        pr = grp * NPAIR + pp
                        for h in range(2):
                            ps = cpsum.tile([128, 2, T], f32, tag="cps")
                            ops = []
                            for cc in range(2):
                                for k in range(5):
                                    ops.append((
                                        t_wc[:, (cc * 5 + k) * 2 + h, :],
                                        t_x[:, cc, 2 * pr:2 * pr + 2,
                                            k:k + T], None))
                            if l == 0:
                                ops.append((
                                    t_wc0e[:, h * 128:(h + 1) * 128],
                                    t_xc[:, 2 * pr:2 * pr + 2, :], None))
                            for j, (lh, rh, dt) in enumerate(ops):
                                mm(ps[:], lh, rh, j == 0, j == len(ops) - 1,
                                   dt=dt)
                            for i in range(2):
                                sl = pp * 2 + i
                                nc.scalar.activation(
                                    out=hraw[h][:, sl, :], in_=ps[:, i, :],
                                    func=AF.Identity,
                                    accum_out=sums[h][:, sl:sl + 1])
                                scr = scr_p.tile([128, T], fp16, tag="sq")
                                nc.vector.scalar_tensor_tensor(
                                    out=scr[:], in0=hraw[h][:, sl, :],
                                    scalar=1.0,
                                    in1=hraw[h][:, sl, :], op0=OP.mult,
                                    op1=OP.mult,
                                    accum_out=qs[h][:, sl:sl + 1])

                    # ---- phase 2: group stats -> A, B per half
                    AB = []
                    for h in range(2):
                        g1 = stps.tile([8, SG], f32, tag="gg")
                        mm(g1[:], t_gind[:], sums[h][:], True, True)
                        g2 = stps.tile([8, SG], f32, tag="gg")
                        mm(g2[:], t_gind[:], qs[h][:], True, True)
                        mean = small_p.tile([8, SG], f32, tag="mean")
                        nc.vector.tensor_scalar_mul(mean[:], g1[:],
                                                    1.0 / (GRP * T))
                        msq = small_p.tile([8, SG], f32, tag="msq")
                        nc.vector.tensor_tensor(out=msq[:], in0=mean[:],
                                                in1=mean[:], op=OP.mult)
                        var = small_p.tile([8, SG], f32, tag="var")
                        nc.vector.scalar_tensor_tensor(
                            out=var[:], in0=g2[:], scalar=1.0 / (GRP * T),
                            in1=msq[:], op0=OP.mult, op1=OP.subtract)
                        sd = small_p.tile([8, SG], f32, tag="sd")
                        nc.scalar.activation(out=sd[:], in_=var[:],
                                             func=AF.Sqrt,
                                             bias=t_eps[:, :1], scale=1.0)
                        rstd = small_p.tile([8, SG], f32, tag="rstd")
                        nc.vector.reciprocal(rstd[:], sd[:])
                        rp = stps.tile([128, SG], f32, tag="gg")
                        mm(rp[:], t_gexp[:], rstd[:], True, True)
                        mp = stps.tile([128, SG], f32, tag="gg")
                        mm(mp[:], t_gexp[:], mean[:], True, True)
                        At = small_p.tile([128, SG], f32, tag="A")
                        nc.vector.tensor_scalar_mul(
                            At[:], rp[:],
                            t_gamma[:, l * 2 + h:l * 2 + h + 1])
                        tmp = small_p.tile([128, SG], f32, tag="tmp")
                        nc.vector.tensor_tensor(out=tmp[:], in0=mp[:],
                                                in1=At[:], op=OP.mult)
                        Bt = small_p.tile([128, SG], f32, tag="B")
                        nc.vector.tensor_scalar(
                            out=Bt[:], in0=tmp[:], scalar1=-1.0,
                            scalar2=t_beta[:, l * 2 + h:l * 2 + h + 1],
                            op0=OP.mult, op1=OP.add)
                        AB.append((At, Bt))

                    # ---- phase 3: normalize+relu, transpose, interp matmul
                    for pp in range(NPAIR):
                        pr = grp * NPAIR + pp
                        s128 = sm_p.tile([128, 2, T], fp16, tag="s128")
                        nc.gpsimd.dma_start(
                            out=s128[:],
                            in_=d_wS[l, 2 * pr:2 * pr + 2, 0:128, :].rearrange(
                                "s t w -> t s w"))
                        s64 = sm_p.tile([64, 2, T], fp16, tag="s64")
                        nc.gpsimd.dma_start(
                            out=s64[:],
                            in_=d_wS[l, 2 * pr:2 * pr + 2, 128:192, :].rearrange(
                                "s t w -> t s w"))
                        for i in range(2):
                            sl = pp * 2 + i
                            sg_ = 2 * pr + i
                            yth = [yt_p.tile([128, 2, 128], fp16,
                                             tag=f"yth{h}", name=f"yth{h}")
                                   for h in range(2)]
                            for h in range(2):
                                At, Bt = AB[h]
                                ytmp = y_p.tile([128, T], fp16, tag="ytmp")
                                nc.vector.tensor_scalar(
                                    out=ytmp[:], in0=hraw[h][:, sl, :],
                                    scalar1=At[:, sl:sl + 1],
                                    scalar2=Bt[:, sl:sl + 1],
                                    op0=OP.mult, op1=OP.add)
                                yb = y_p.tile([128, T], fp16, tag="yb")
                                nc.vector.tensor_scalar_max(
                                    yb[:], ytmp[:], 0.0)
                                ptp = tpsum.tile([128, 2, 128], fp16,
                                                 tag="tp", name="ptp")
                                nc.tensor.transpose(
                                    out=ptp[:, 0, :], in_=yb[:, 0:128],
                                    identity=t_id128[:])
                                nc.tensor.transpose(
                                    out=ptp[0:64, 1, :], in_=yb[:, 128:192],
                                    identity=t_id128[:])
                                nc.vector.tensor_copy(
                                    out=yth[h][:], in_=ptp[:, :, :])
                            sout = sops.tile([128, 2, T], f32, tag="so",
                                             name="sout")
                            for ch in range(2):
                                mm(sout[:, ch, :], yth[ch][:, 0, :],
                                   s128[:, i, :], True, False)
                                mm(sout[:, ch, :], yth[ch][0:64, 1, :],
                                   s64[:, i, :], False, True)
                            nc.vector.tensor_copy(
                                out=t_x[:, :, sg_, 2:194],
                                in_=sout[:, :, :])
                            if l == 2:
                                # rearrange to [c, cc, t, s] for the LSTM xw
                                # matmuls on the otherwise-idle gpsimd engine
                                nc.gpsimd.tensor_copy(
                                    out=t_xT[:, :, :, sg_],
                                    in_=t_x[:, :, sg_, 2:194])

                if probe_layer == l:
                    for h in range(2):
                        nc.gpsimd.dma_start(out=d_probe[h, :, :, :],
                                            in_=t_x[:, h, :, :])

        # ======================= biLSTM (gate-major) =======================
        # state tiles [64 part=(dir,unit), 64 cols=sample]; gates in PSUM
        # blocks [64, LBLK, 4(gate i,f,g,o), 64], one PSUM bank per block.
        lsb = es.enter_context(tc.tile_pool(name="lstm_sbuf", bufs=1))
        t_SIG = lsb.tile([64, 4, 64], fp16, name="sig")  # sig(i,f,2g,o)
        t_Q = lsb.tile([64, 64], fp16, name="qq")        # (sig2g-.5)*si
        t_R = lsb.tile([64, 64], fp16, name="rr")        # sf*c
        t_C = lsb.tile([64, 64], fp16, name="cc")        # cell state
        t_TC = lsb.tile([64, 64], fp16, name="tc")       # tanh(c)
        t_HT = lsb.tile([65, 64], fp16, name="ht")       # h; row 64 = ones
        t_OUT = lsb.tile([64, NT_OUT, 64], f32, name="outt")
        nc.vector.memset(t_C[:], 0.0)
        nc.vector.memset(t_HT[0:64, :], 0.0)
        nc.vector.memset(t_HT[64:65, :], 1.0)

        with tc.tile_pool(name="lpsum", bufs=2, space="PSUM") as lpsum, \
             tc.tile_pool(name="ltp", bufs=2, space="PSUM") as ltp:

            def xw_half(blk, half, G=None):
                """xw preacts for gates [2*half, 2*half+2) of one block.
                start=True clears the full bank row for the partitions the
                matmul writes -> one start per direction, on its first
                matmul (half 0)."""
                t0 = blk * LBLK
                if G is None:
                    G = lpsum.tile([64, LBLK, 4, 64], f32, tag="xw",
                                   name="xw")
                for g4 in range(2 * half, 2 * half + 2):
                    for cc in range(2):
                        # dir f reads time t0..t0+LBLK-1 (contiguous (t, s))
                        rf = t_xT[:, cc, t0:t0 + LBLK, :]
                        mm(G[0:32, :, g4, :], t_wihG[:, g4, cc, 0:32], rf,
                           g4 == 2 * half == 0 and cc == 0, False)
                        # dir b reads time 191-t0 downward (negative t stride)
                        base = t_xT[:, cc, 0:LBLK, :]
                        rb = bass.AP(tensor=base.tensor,
                                     offset=base.offset + (191 - t0) * S,
                                     ap=[base.ap[0], [-S, LBLK], [1, S]])
                        mm(G[32:64, :, g4, :], t_wihG[:, g4, cc, 32:64], rb,
                           g4 == 2 * half == 0 and cc == 0, False)
                return G

            xwp = [xw_half(0, 1, xw_half(0, 0)), None]
            t_OUTT = lsb.tile([64, NT_OUT, 64], fp16, name="outth")
            if probe_layer == 4:
                t_gdbg = lsb.tile([64, LBLK, 4, 64], f32, name="gdbg")
                nc.vector.tensor_copy(out=t_gdbg[:], in_=xwp[0][:, :, :, :])
                nc.gpsimd.dma_start(out=d_probe[0, 0:64, 0:LBLK * 4, 0:64],
                                    in_=t_gdbg[:, :, :, :])

            for g in range(T):
                blk, j = g // LBLK, g % LBLK
                G = xwp[blk % 2]
                # i, f, g rec matmuls first; sigmoid(i,f,2g) can then start
                # while the o-gate matmul + sigmoid run off the critical path
                for g4 in range(3):
                    mm(G[:, j, g4, :], t_whhG[:, g4, :], t_HT[:],
                       False, True, skip_group_check=True)
                nc.scalar.activation(
                    out=t_SIG[:, 0:3, :], in_=G[:, j, 0:3, :],
                    func=AF.Sigmoid)
                mm(G[:, j, 3, :], t_whhG[:, 3, :], t_HT[:],
                   False, True, skip_group_check=True)
                nc.scalar.activation(
                    out=t_SIG[:, 3, :], in_=G[:, j, 3, :],
                    func=AF.Sigmoid)
                # q = (sig(2g) - 0.5) * sig(i)  [= sig(i)*tanh(g)/2]
                nc.vector.scalar_tensor_tensor(
                    out=t_Q[:], in0=t_SIG[:, 2, :], scalar=0.5,
                    in1=t_SIG[:, 0, :], op0=OP.subtract, op1=OP.mult)
                # r = sig(f) * c  (gpsimd, parallel with q)
                nc.gpsimd.tensor_tensor(out=t_R[:], in0=t_SIG[:, 1, :],
                                        in1=t_C[:], op=OP.mult)
                # c = 2*q + r
                nc.vector.scalar_tensor_tensor(
                    out=t_C[:], in0=t_Q[:], scalar=2.0,
                    in1=t_R[:], op0=OP.mult, op1=OP.add)
                nc.scalar.activation(out=t_TC[:], in_=t_C[:],
                                     func=AF.Tanh)
                nc.vector.tensor_tensor(out=t_HT[0:64, :],
                                        in0=t_SIG[:, 3, :],
                                        in1=t_TC[:], op=OP.mult)
                if g % FREQ == FREQ - 1:
                    nc.vector.tensor_copy(
                        out=t_OUTT[0:32, g // FREQ, :], in_=t_HT[0:32, :])
                    nc.vector.tensor_copy(
                        out=t_OUTT[32:64, (T - 1 - g) // FREQ, :],
                        in_=t_HT[32:64, :])
                if j == 0 and blk + 1 < NBLK:
                    xwp[(blk + 1) % 2] = xw_half(blk + 1, 0)
                if j == 1 and blk + 1 < NBLK:
                    xw_half(blk + 1, 1, xwp[(blk + 1) % 2])
                if probe_layer == 5 and g == 0:
                    nc.gpsimd.dma_start(out=d_probe[0, 0:64, 0, 0:256],
                                        in_=t_SIG[:, :, :])
                    nc.gpsimd.dma_start(out=d_probe[0, 0:64, 3, 0:64],
                                        in_=t_TC[:, :])
                    nc.gpsimd.dma_start(out=d_probe[0, 0:64, 4, 0:64],
                                        in_=t_HT[:, :])
            # post-loop: transpose the collected h tiles to [sample, du]
            for k in range(NT_OUT):
                pht = ltp.tile([64, 64], fp16, tag="pht", name="pht")
                nc.tensor.transpose(out=pht[:], in_=t_OUTT[:, k, :],
                                    identity=t_id128[0:64, 0:64])
                nc.vector.tensor_copy(out=t_OUT[:, k, :], in_=pht[:])

        nc.sync.dma_start(out=d_out[:, :, :], in_=t_OUT[:])

    nc.compile()
    return nc


def _get_nc(probe_layer=-1):
    key = ("nc", probe_layer)
    if key not in _cache:
        _cache[key] = _build(probe_layer)
    return _cache[key]


def run_on_cores(inputs, probe_layer=-1, trace=False):
    """Build (cached), run on 8 cores; returns (results, BassKernelResults)."""
    from concourse.bass_utils import run_bass_kernel_spmd

    nc = _get_nc(probe_layer)
    in_maps = _prep_host(inputs)
    last_exc = None
    for _ in range(3):
        try:
            res = run_bass_kernel_spmd(nc, in_maps,
                                       core_ids=list(range(N_CORES)),
                                       trace=trace)
            return res
        except Exception as e:  # transient NRT errors happen; retry
            last_exc = e
    raise last_exc


def assemble_output(res):
    out = np.zeros((B, NT_OUT, 64), np.float32)
    for core in range(N_CORES):
        s0 = core * S
        out[s0:s0 + S] = res.results[core]["out"]
    return out


def kernel(**inputs):
    res = run_on_cores(inputs)
    return assemble_output(res)

